# revision 10
# baseline (speedup 1.0000x reference)
"""Trainium2 Bass kernel for MultiHeadAttention with RoPE.

Problem: B=2, L=2048, d_model=1024, 16 heads, d_k=64, fp32 in/out.

Sharding (8 cores): tensor-parallel over heads — core c owns heads
{2c, 2c+1}, i.e. a 128-wide slice of the projection output dims.  Every
core reads the full q/k/v activations (transposed + bf16 on host), its
own 128-row slice of Wq/Wk/Wv (pre-transposed) and the matching 128
columns of Wo.  Each core computes its heads' attention output and the
partial d_model-sized output projection; the host sums the 8 partials
and adds bo (+ Wo @ bv, folded out of the device program).

Per-core pipeline (all matmuls bf16, fp32 PSUM accumulation):
  1. QKV projections:  qh.T = WqT.T @ q.T  laid out [128 head-dims, 4096 tok]
  2. RoPE on q,k via partition-shifted DMA copy + 3 DVE ops; the 1/sqrt(dk)
     scale and the rotate-half sign are folded into host-built cos/sin tables
  3. scores.T tiles [kt 128, qt 512] = kh2 (block-diag stationary, K=128) @ qh
  4. exp on ScalarE (no max-subtract: scores ~ N(0,1)), output bf16 -> SBUF
  5. ctx accumulation [128, qt] via block-diag vh2 stationary; a parallel
     ones2 [128, 2] matmul accumulates both heads' softmax denominators
     into a [2, 1024] PSUM tile
  6. normalize: DVE reciprocal of the denominators, then a K=2 PE matmul
     broadcasts them across 128 partitions; one DVE multiply produces the
     normalized ctx (flash-style deferred normalization)
  7. out_partial[tok, 1024] = ctx (stationary) @ WoT slice, evictions
     alternate DVE/GpSimd, stores batched 4 tiles per DMA

Scheduling: bulk input loads and output stores ride the sync-engine DMA
queue; compute-dependent small moves (rope rotate, vh2 placement) use the
scalar-engine hardware DGE queue so input prefetch is never blocked.
Batch-1 input DMA is issued before batch-0 attention so it streams in the
background; projection compute for batch 1 and out_proj calls are
interleaved to keep the PE warm (HAM clock gate) end to end.
"""

import os
import numpy as np
import ml_dtypes

import concourse.bass as bass
import concourse.mybir as mybir
import concourse.tile as tile
from concourse import bacc
from concourse.bass_utils import run_bass_kernel_spmd

BF = mybir.dt.bfloat16
F32 = mybir.dt.float32
AF = mybir.ActivationFunctionType

NCORES = 8
B = 2
L = 2048
D = 1024          # d_model
H = 16            # heads
DK = 64           # head dim
HPC = H // NCORES  # heads per core = 2
PD = HPC * DK      # projection dims per core = 128
TOK = B * L        # 4096 tokens
P = 128

ROPE_BASE = 10000.0


def build_nc(debug_dumps=False):
    """Build the single-core Bass program (SPMD: same program, per-core data)."""
    from contextlib import ExitStack

    nc = bacc.Bacc("TRN2", target_bir_lowering=False, debug=False)

    # ---- DRAM I/O ----
    qT = nc.dram_tensor("qT", [D, TOK], BF, kind="ExternalInput").ap()
    kT = nc.dram_tensor("kT", [D, TOK], BF, kind="ExternalInput").ap()
    vT = nc.dram_tensor("vT", [D, TOK], BF, kind="ExternalInput").ap()
    wqT = nc.dram_tensor("wqT", [D, PD], BF, kind="ExternalInput").ap()
    wkT = nc.dram_tensor("wkT", [D, PD], BF, kind="ExternalInput").ap()
    wvT = nc.dram_tensor("wvT", [D, PD], BF, kind="ExternalInput").ap()
    woT = nc.dram_tensor("woT", [PD, D], BF, kind="ExternalInput").ap()
    bq_d = nc.dram_tensor("bq", [PD, 1], F32, kind="ExternalInput").ap()
    bk_d = nc.dram_tensor("bk", [PD, 1], F32, kind="ExternalInput").ap()
    cos_q = nc.dram_tensor("cos_q", [P, L], BF, kind="ExternalInput").ap()
    sin_q = nc.dram_tensor("sin_q", [P, L], BF, kind="ExternalInput").ap()
    cos_k = nc.dram_tensor("cos_k", [P, L], BF, kind="ExternalInput").ap()
    sin_k = nc.dram_tensor("sin_k", [P, L], BF, kind="ExternalInput").ap()
    ones2_d = nc.dram_tensor("ones2", [P, 2], BF, kind="ExternalInput").ap()
    sel_d = nc.dram_tensor("sel", [2, P], BF, kind="ExternalInput").ap()
    outp = nc.dram_tensor("outp", [TOK, D], BF, kind="ExternalOutput").ap()

    with tile.TileContext(nc) as tc, ExitStack() as ctx:
        const = ctx.enter_context(tc.tile_pool(name="const", bufs=1))
        persist = ctx.enter_context(tc.tile_pool(name="persist", bufs=1))
        stage = ctx.enter_context(tc.tile_pool(name="stage", bufs=10))
        raws = ctx.enter_context(tc.tile_pool(name="raws", bufs=2))
        rots = ctx.enter_context(tc.tile_pool(name="rots", bufs=2))
        expp = ctx.enter_context(tc.tile_pool(name="expp", bufs=3))
        outs = ctx.enter_context(tc.tile_pool(name="outs", bufs=2))
        smalls = ctx.enter_context(tc.tile_pool(name="smalls", bufs=4))
        mmp = ctx.enter_context(tc.tile_pool(name="mmp", bufs=2, space="PSUM"))
        ctxp = ctx.enter_context(tc.tile_pool(name="ctxp", bufs=1, space="PSUM"))
        rsp = ctx.enter_context(tc.tile_pool(name="rsp", bufs=1, space="PSUM"))
        vhtp = ctx.enter_context(tc.tile_pool(name="vhtp", bufs=1))

        # ---- prewarm the ScalarE exp table set during the initial DMA wait
        warm_in = smalls.tile([1, 1], F32, name="warm_in")
        warm_out = smalls.tile([1, 1], F32, name="warm_out")
        nc.vector.memset(warm_in[:], 0.0)
        nc.scalar.activation(warm_out[:], warm_in[:], AF.Exp)

        # ---- constants into SBUF (phase order so the first projection's
        # matmuls aren't queued behind all of the const DMA) ----
        def load_w(name, w_d):
            w_sb = const.tile([P, 8 * P], BF, name=name)
            nc.sync.dma_start(
                w_sb.rearrange("p (a m) -> p a m", a=8),
                w_d.rearrange("(a p) m -> p a m", p=P),
            )
            return w_sb

        def load_c(name, t_d, shape):
            t_sb = const.tile([P, shape], BF, name=name)
            nc.sync.dma_start(t_sb[:], t_d[:])
            return t_sb

        def load_b(name, b_d):
            b_sb = const.tile([P, 1], F32, name=name)
            nc.sync.dma_start(b_sb[:], b_d[:])
            return b_sb

        wq_sb = load_w("wq_sb", wqT)
        bq_sb = load_b("bq_sb", bq_d)
        cq_sb = load_c("cq_sb", cos_q, L)
        sq_sb = load_c("sq_sb", sin_q, L)

        # persistent activations
        qq_sb = persist.tile([P, TOK], BF)   # roped q-heads  [128 dims, 4096 tok]
        kk_sb = persist.tile([P, TOK], BF)   # roped k-heads
        vh_sb = persist.tile([P, TOK], BF)   # v-heads (dims-major)
        ctx_sb = persist.tile([P, TOK], BF)  # normalized attention ctx
        # Block-diagonal attention operands (both heads packed into K=128 so
        # the PE array runs fully occupied and the HAM clock-gate opens to
        # 2.4 GHz — K=64 matmul streams were measured to stay at 1.2 GHz):
        #   kh2[b]: 32 chunk tiles [128, 128]; chunk c is
        #           [[kh_h0[d, ktA] , 0], [0, kh_h1[d, ktA]]], ktA = 64 tokens
        #   vh2[b]: 32 chunk tiles [128, 128]; chunk c is
        #           [[vh_h0[ktA, d] , 0], [0, vh_h1[ktA, d]]]
        kh2 = [persist.tile([P, 32 * P], BF, name=f"kh2_{b}") for b in range(B)]
        vh2 = [persist.tile([P, 32 * P], BF, name=f"vh2_{b}") for b in range(B)]
        for t in kh2 + vh2:
            nc.gpsimd.memset(t[:], 0.0)
        # ones2 [128, 2]: col 0 sums h0 rows (kt 0..63), col 1 sums h1 rows.
        # M=2 keeps the LDWEIGHTS trivial; denominators land on PSUM
        # partitions 0/1 where the DVE can reach them directly.
        # sel [2, 128]: K=2 stationary that broadcasts the per-head
        # reciprocal rows onto partitions 0..63 / 64..127.
        # Both are host-built constants (on-device memsets can't start at
        # partition 1).
        ones2 = const.tile([P, 2], BF)
        nc.sync.dma_start(ones2[:], ones2_d[:])
        sel_sb = const.tile([2, P], BF)
        nc.sync.dma_start(sel_sb[:], sel_d[:])

        # ---------- phase helpers ----------
        def load_x(x_d, g):
            """Queue the 8 input stage tiles for token half g (sync queue)."""
            xts = []
            for kt in range(8):
                xt = stage.tile([P, L], BF, name="xstage", tag="stage")
                nc.sync.dma_start(
                    xt[:], x_d[kt * P:(kt + 1) * P, g * L:(g + 1) * L])
                xts.append(xt)
            return xts

        def proj_compute(xts, w_sb, bias_sb, g, dst_sb, cos_sb=None, sin_sb=None):
            """Project token half g (2048 tokens) and optionally apply RoPE.

            Writes dst_sb[:, g*2048:(g+1)*2048] (bf16).
            """
            ps = [mmp.tile([P, 1024], F32, name=f"pj{g}_{half}", tag="mm")
                  for half in range(2)]
            for kt in range(8):
                xt = xts[kt]
                for half in range(2):
                    for nb in range(2):
                        c0 = half * 1024 + nb * 512
                        nc.tensor.matmul(
                            ps[half][:, nb * 512:(nb + 1) * 512],
                            lhsT=w_sb[:, kt * P:(kt + 1) * P],
                            rhs=xt[:, c0:c0 + 512],
                            start=(kt == 0), stop=(kt == 7),
                        )
            if cos_sb is None:
                # no rope (v): evict straight to destination (no bias: bv is
                # folded into the host-side output bias via Wo @ bv)
                for half in range(2):
                    nc.scalar.activation(
                        dst_sb[:, g * L + half * 1024: g * L + (half + 1) * 1024],
                        ps[half][:], AF.Identity)
                return
            raw = raws.tile([P, L], BF, name="raw", tag="raw")
            for half in range(2):
                nc.scalar.activation(
                    raw[:, half * 1024:(half + 1) * 1024],
                    ps[half][:], AF.Identity, bias=bias_sb[:])
            rot = rots.tile([P, L], BF, name="rot", tag="rot")
            # rotate-half as partition-block moves (sign folded into sin
            # table); scalar-queue DGE so the sync queue stays free for
            # input prefetch
            for h in range(HPC):
                r0 = h * DK
                nc.scalar.dma_start(rot[r0:r0 + 32, :], raw[r0 + 32:r0 + 64, :])
                nc.scalar.dma_start(rot[r0 + 32:r0 + 64, :], raw[r0:r0 + 32, :])
            dst = dst_sb[:, g * L:(g + 1) * L]
            nc.vector.tensor_mul(raw[:], raw[:], cos_sb[:])
            nc.vector.tensor_mul(rot[:], rot[:], sin_sb[:])
            nc.vector.tensor_add(dst, raw[:], rot[:])

        def build_kv2(b):
            """Fill this batch's block-diagonal kh2/vh2 operand buffers."""
            # kh2: both copies are partition-aligned (h1 dims already live on
            # partitions 64..127 of kk_sb)
            kh2_r = kh2[b].rearrange("p (c u) -> p c u", u=P)
            kk_b = kk_sb[:, b * L:(b + 1) * L]
            nc.vector.tensor_copy(
                kh2_r[0:DK, :, 0:DK],
                kk_b[0:DK, :].rearrange("p (c u) -> p c u", u=DK))
            nc.vector.tensor_copy(
                kh2_r[DK:P, :, DK:P],
                kk_b[DK:P, :].rearrange("p (c u) -> p c u", u=DK))
            # vh2 needs [token, dim] tiles: PE-transpose 128-token tiles of
            # vh_sb, then 4 strided SBUF->SBUF DMAs place the 64-token
            # half-tiles into their diagonal blocks
            vht = vhtp.tile([P, 16 * P], BF, name="vht", tag="vht")
            for t in range(16):
                pt = mmp.tile([P, P], BF, name="pt", tag="mm")
                nc.tensor.transpose(
                    pt[:], vh_sb[:, b * L + t * P: b * L + (t + 1) * P],
                    ident[:])
                nc.vector.tensor_copy(vht[:, t * P:(t + 1) * P], pt[:])
            vht_r = vht.rearrange("p (t u) -> p t u", u=P)
            vh2_r = vh2[b].rearrange("p (t x) -> p t x", x=2 * P)
            # even chunks come from vht rows 0..63, odd chunks from 64..127
            nc.scalar.dma_start(vh2_r[0:DK, :, 0:DK], vht_r[0:DK, :, 0:DK])
            nc.scalar.dma_start(vh2_r[0:DK, :, 2 * DK:3 * DK], vht_r[DK:P, :, 0:DK])
            nc.scalar.dma_start(vh2_r[DK:P, :, DK:2 * DK], vht_r[0:DK, :, DK:P])
            nc.scalar.dma_start(vh2_r[DK:P, :, 3 * DK:4 * DK], vht_r[DK:P, :, DK:P])

        def attention(b, q2):
            """Both heads at once via block-diagonal K=128 matmuls.

            Scores chunk c: sc[0:64]  = scoresT_h0[ktA, qt],
                            sc[64:128] = scoresT_h1[ktA, qt].
            ctx accumulates [h0 dims | h1 dims, qt]; a parallel ones2 [128,2]
            matmul accumulates both heads' softmax denominators in a [2, qt]
            PSUM tile.  Software-pipelined so the PE never waits on the exp.
            """
            qs = qq_sb[:, b * L + q2 * 1024: b * L + (q2 + 1) * 1024]
            cp = ctxp.tile([P, 1024], F32, name="cp", tag="ctx")
            rs = rsp.tile([2, 1024], F32, name="rs", tag="rs")
            ex_prev = None
            for c in range(33):
                ex_cur = None
                if c < 32:
                    sc = mmp.tile([P, 1024], F32, name="sc", tag="mm")
                    for nb in range(2):
                        nc.tensor.matmul(
                            sc[:, nb * 512:(nb + 1) * 512],
                            lhsT=kh2[b][:, c * P:(c + 1) * P],
                            rhs=qs[:, nb * 512:(nb + 1) * 512],
                            start=True, stop=True, skip_group_check=True,
                        )
                    ex_cur = expp.tile([P, 1024], BF, name="ex", tag="exp")
                    nc.scalar.activation(ex_cur[:], sc[:], AF.Exp)
                if c >= 1:
                    cpv = c - 1
                    for nb in range(2):
                        sl = slice(nb * 512, (nb + 1) * 512)
                        nc.tensor.matmul(
                            cp[:, sl], lhsT=vh2[b][:, cpv * P:(cpv + 1) * P],
                            rhs=ex_prev[:, sl],
                            start=(cpv == 0), stop=(cpv == 31),
                            skip_group_check=True,
                        )
                    for nb in range(2):
                        sl = slice(nb * 512, (nb + 1) * 512)
                        nc.tensor.matmul(
                            rs[:, sl], lhsT=ones2[:], rhs=ex_prev[:, sl],
                            start=(cpv == 0), stop=(cpv == 31),
                            skip_group_check=True,
                        )
                ex_prev = ex_cur
            # normalize: reciprocal of the two denominator rows, broadcast
            # across partitions with a K=2 matmul, one DVE multiply.
            rsum = smalls.tile([2, 1024], F32, name="rsum", tag="rsum", bufs=2)
            nc.vector.tensor_copy(rsum[:], rs[:])
            rec = smalls.tile([2, 1024], F32, name="rec", tag="rec", bufs=2)
            nc.vector.reciprocal_approx_fast(rec[:], rsum[:])
            rec_bf = smalls.tile([2, 1024], BF, name="recbf", tag="recbf", bufs=2)
            nc.vector.tensor_copy(rec_bf[:], rec[:])
            bcs = rsp.tile([P, 1024], F32, name="bcs", tag="rs")
            for nb in range(2):
                sl = slice(nb * 512, (nb + 1) * 512)
                nc.tensor.matmul(
                    bcs[:, sl], lhsT=sel_sb[:], rhs=rec_bf[:, sl],
                    start=True, stop=True, skip_group_check=True,
                )
            craw = smalls.tile([P, 1024], BF, name="craw", tag="craw", bufs=2)
            nc.vector.tensor_copy(craw[:], cp[:])
            c0 = b * L + q2 * 1024
            nc.vector.tensor_mul(ctx_sb[:, c0:c0 + 1024], craw[:], bcs[:])

        def out_proj(b, q2):
            ob = None
            for i, tb in enumerate(range(q2 * 8, q2 * 8 + 8)):
                t0 = b * L + tb * P
                if i % 4 == 0:
                    ob = outs.tile([P, 4 * D], BF, name="ob", tag="out")
                po = mmp.tile([P, D], F32, name="po", tag="mm")
                for nb in range(2):
                    nc.tensor.matmul(
                        po[:, nb * 512:(nb + 1) * 512],
                        lhsT=ctx_sb[:, t0:t0 + P],
                        rhs=wo_sb[:, nb * 512:(nb + 1) * 512],
                        start=True, stop=True, skip_group_check=True,
                    )
                dst = ob[:, (i % 4) * D:(i % 4 + 1) * D]
                nc.vector.tensor_copy(dst, po[:])
                if i % 4 == 3:
                    t00 = b * L + (tb - 3) * P
                    nc.sync.dma_start(
                        outp[t00:t00 + 4 * P, :].rearrange(
                            "(a p) d -> p a d", p=P),
                        ob.rearrange("p (a d) -> p a d", a=4))

        # ---------- program ----------
        xq0 = load_x(qT, 0)
        proj_compute(xq0, wq_sb, bq_sb, 0, qq_sb, cq_sb, sq_sb)
        wk_sb = load_w("wk_sb", wkT)
        bk_sb = load_b("bk_sb", bk_d)
        ck_sb = load_c("ck_sb", cos_k, L)
        sk_sb = load_c("sk_sb", sin_k, L)
        xk0 = load_x(kT, 0)
        proj_compute(xk0, wk_sb, bk_sb, 0, kk_sb, ck_sb, sk_sb)
        wv_sb = load_w("wv_sb", wvT)
        ident = const.tile([P, P], BF)
        from concourse.masks import make_identity
        make_identity(nc, ident[:])
        wo_sb = const.tile([P, D], BF)
        nc.sync.dma_start(wo_sb[:], woT[:])
        xv0 = load_x(vT, 0)
        proj_compute(xv0, wv_sb, None, 0, vh_sb)
        build_kv2(0)
        # queue batch-1 input DMA now: it streams on the sync queue while
        # the PE runs batch-0 attention
        xq1 = load_x(qT, 1)
        xk1 = load_x(kT, 1)
        xv1 = load_x(vT, 1)
        attention(0, 0)
        attention(0, 1)
        out_proj(0, 0)
        proj_compute(xq1, wq_sb, bq_sb, 1, qq_sb, cq_sb, sq_sb)
        proj_compute(xk1, wk_sb, bk_sb, 1, kk_sb, ck_sb, sk_sb)
        out_proj(0, 1)
        proj_compute(xv1, wv_sb, None, 1, vh_sb)
        build_kv2(1)
        attention(1, 0)
        attention(1, 1)
        out_proj(1, 0)
        out_proj(1, 1)

    return nc


def _rope_tables():
    """Host-built RoPE tables, transposed to [d, t], 2 heads stacked.

    sin is sign-folded for the rotate-half convention; q tables carry the
    1/sqrt(dk) attention scale.
    """
    inv_freq = 1.0 / (ROPE_BASE ** (np.arange(0, DK, 2, dtype=np.float64) / DK))
    t = np.arange(L, dtype=np.float64)
    ang = np.outer(t, inv_freq)               # [L, 32]
    emb = np.concatenate([ang, ang], axis=1)  # [L, 64]
    cos = np.cos(emb).T.astype(np.float32)    # [64, L]
    sin = np.sin(emb).T.astype(np.float32)
    sin_folded = sin.copy()
    sin_folded[:32] *= -1.0
    scale = 1.0 / np.sqrt(DK)
    cos2 = np.concatenate([cos, cos], axis=0)                # [128, L]
    sin2 = np.concatenate([sin_folded, sin_folded], axis=0)  # [128, L]
    bf = ml_dtypes.bfloat16
    return (
        (cos2 * scale).astype(bf), (sin2 * scale).astype(bf),
        cos2.astype(bf), sin2.astype(bf),
    )


_NC_CACHE = {}


def _get_nc():
    if "nc" not in _NC_CACHE:
        nc = build_nc()
        nc.finalize()
        _NC_CACHE["nc"] = nc
    return _NC_CACHE["nc"]


def _in_maps(q, k, v, Wq, bq, Wk, bk, Wv, Wo):
    bf = ml_dtypes.bfloat16
    qT = np.ascontiguousarray(q.reshape(TOK, D).T).astype(bf)
    kT = np.ascontiguousarray(k.reshape(TOK, D).T).astype(bf)
    vT = np.ascontiguousarray(v.reshape(TOK, D).T).astype(bf)
    cos_q, sin_q, cos_k, sin_k = _rope_tables()
    ones2 = np.zeros((P, 2), bf)
    ones2[0:DK, 0] = 1
    ones2[DK:P, 1] = 1
    sel = np.zeros((2, P), bf)
    sel[0, 0:DK] = 1
    sel[1, DK:P] = 1
    in_maps = []
    for c in range(NCORES):
        hs = slice(c * PD, (c + 1) * PD)
        in_maps.append({
            "qT": qT, "kT": kT, "vT": vT,
            "wqT": np.ascontiguousarray(Wq[hs, :].T).astype(bf),
            "wkT": np.ascontiguousarray(Wk[hs, :].T).astype(bf),
            "wvT": np.ascontiguousarray(Wv[hs, :].T).astype(bf),
            "woT": np.ascontiguousarray(Wo[:, hs].T).astype(bf),
            "bq": np.asarray(bq[hs], np.float32).reshape(PD, 1),
            "bk": np.asarray(bk[hs], np.float32).reshape(PD, 1),
            "cos_q": cos_q, "sin_q": sin_q, "cos_k": cos_k, "sin_k": sin_k,
            "ones2": ones2, "sel": sel,
        })
    return in_maps


def kernel(q, k, v, Wq, bq, Wk, bk, Wv, bv, Wo, bo):
    assert q.shape == (B, L, D) and k.shape == (B, L, D) and v.shape == (B, L, D)
    in_maps = _in_maps(q, k, v, Wq, bq, Wk, bk, Wv, Wo)
    nc = _get_nc()
    res = run_bass_kernel_spmd(nc, in_maps, list(range(NCORES)))
    out = np.zeros((TOK, D), np.float64)
    for r in res.results:
        out += r["outp"].astype(np.float64)
    # bv never touches the device: since the attention weights sum to 1,
    # its contribution is the constant vector Wo @ bv
    out += np.asarray(bo, np.float64)[None, :]
    out += (np.asarray(Wo, np.float64) @ np.asarray(bv, np.float64))[None, :]
    return out.astype(np.float32).reshape(B, L, D)


# revision 12
# speedup vs baseline: 1.0911x; 1.0911x over previous
"""Trainium2 Bass kernel for MultiHeadAttention with RoPE.

Problem: B=2, L=2048, d_model=1024, 16 heads, d_k=64, fp32 in/out.

Sharding (8 cores): tensor-parallel over heads — core c owns heads
{2c, 2c+1}, i.e. a 128-wide slice of the projection output dims.  Every
core reads the full q/k/v activations (transposed + bf16 on host), its
own 128-row slice of Wq/Wk/Wv (pre-transposed) and the matching 128
columns of Wo.  Each core computes its heads' attention output and the
partial d_model-sized output projection; the host sums the 8 partials
and adds bo (+ Wo @ bv, folded out of the device program).

Per-core pipeline (all matmuls bf16, fp32 PSUM accumulation):
  1. q/k projections:  qh.T = WqT.T @ q.T  laid out [128 head-dims, 2048 tok]
     per batch, RoPE applied via partition-shifted DMA copy + 3 DVE ops
     (1/sqrt(dk) scale and rotate-half sign folded into host tables)
  2. v projection runs token-major (stationary = x tile, moving = WvT
     chunk) so its PSUM tiles are already [token, dim] and evict straight
     into the block-diagonal vh2 operand with plain DVE casts — no PE
     transposes, no staging buffer, no serial placement phase
  3. scores.T chunks [kt-pack 128, qt 512] = kh2 (block-diag stationary,
     K=128 so the HAM clock-gate opens to 2.4 GHz) @ qh
  4. exp on ScalarE (no max-subtract: scores ~ N(0,1)), bf16 out
  5. ctx accumulation via block-diag vh2 stationary; parallel [128, 2]
     ones matmuls accumulate both heads' softmax denominators in [2, qt]
     PSUM.  The diagonal packing is *skewed* (odd chunks anti-diagonal)
     so the token-major v eviction never crosses partitions; the ones
     stationary alternates parity to keep head sums separate.
  6. normalize: DVE reciprocal of the denominators, K=2 PE matmul
     broadcast across partitions, one DVE multiply (flash-style deferred
     normalization)
  7. out_partial[tok, 1024] = ctx (stationary) @ WoT slice; evictions on
     ScalarE (DVE is busy with rope), stores batched 4 tiles per DMA

Scheduling: bulk loads/stores ride the sync DMA queue; compute-dependent
moves (rope rotate, table duplication) use the scalar-engine hardware DGE
queue.  Batch-1 input DMA is queued before batch-0 attention and batch-1
projections are interleaved *between* attention phases so the 10-buffer
stage pool keeps recycling and the DMA never goes idle.  Warm-up matmuls
at t~2us open the HAM clock gate before the first projection.
"""

import os
import numpy as np
import ml_dtypes

import concourse.bass as bass
import concourse.mybir as mybir
import concourse.tile as tile
from concourse import bacc
from concourse.bass_utils import run_bass_kernel_spmd

BF = mybir.dt.bfloat16
F32 = mybir.dt.float32
AF = mybir.ActivationFunctionType

NCORES = 8
B = 2
L = 2048
D = 1024          # d_model
H = 16            # heads
DK = 64           # head dim
HPC = H // NCORES  # heads per core = 2
PD = HPC * DK      # projection dims per core = 128
TOK = B * L        # 4096 tokens
P = 128

ROPE_BASE = 10000.0


def build_nc(debug_dumps=False):
    """Build the single-core Bass program (SPMD: same program, per-core data)."""
    from contextlib import ExitStack

    nc = bacc.Bacc("TRN2", target_bir_lowering=False, debug=False)

    # ---- DRAM I/O ----
    qT = nc.dram_tensor("qT", [D, TOK], BF, kind="ExternalInput").ap()
    kT = nc.dram_tensor("kT", [D, TOK], BF, kind="ExternalInput").ap()
    vT = nc.dram_tensor("vT", [D, TOK], BF, kind="ExternalInput").ap()
    wqT = nc.dram_tensor("wqT", [D, PD], BF, kind="ExternalInput").ap()
    wkT = nc.dram_tensor("wkT", [D, PD], BF, kind="ExternalInput").ap()
    wvT = nc.dram_tensor("wvT", [D, PD], BF, kind="ExternalInput").ap()
    woT = nc.dram_tensor("woT", [PD, D], BF, kind="ExternalInput").ap()
    bq_d = nc.dram_tensor("bq", [PD, 1], F32, kind="ExternalInput").ap()
    bk_d = nc.dram_tensor("bk", [PD, 1], F32, kind="ExternalInput").ap()
    # RoPE tables ship as the unique 64 head-dim rows; duplicated on-chip
    cos_q = nc.dram_tensor("cos_q", [DK, L], BF, kind="ExternalInput").ap()
    sin_q = nc.dram_tensor("sin_q", [DK, L], BF, kind="ExternalInput").ap()
    cos_k = nc.dram_tensor("cos_k", [DK, L], BF, kind="ExternalInput").ap()
    sin_k = nc.dram_tensor("sin_k", [DK, L], BF, kind="ExternalInput").ap()
    # ones2 [128, 4]: cols 0/1 = even-chunk head-0/head-1 row selectors,
    # cols 2/3 = odd-chunk selectors (skewed packing swaps the halves)
    ones2_d = nc.dram_tensor("ones2", [P, 4], BF, kind="ExternalInput").ap()
    sel_d = nc.dram_tensor("sel", [2, P], BF, kind="ExternalInput").ap()
    outp = nc.dram_tensor("outp", [TOK, D], BF, kind="ExternalOutput").ap()

    with tile.TileContext(nc) as tc, ExitStack() as ctx:
        const = ctx.enter_context(tc.tile_pool(name="const", bufs=1))
        persist = ctx.enter_context(tc.tile_pool(name="persist", bufs=1))
        stage = ctx.enter_context(tc.tile_pool(name="stage", bufs=10))
        raws = ctx.enter_context(tc.tile_pool(name="raws", bufs=2))
        rots = ctx.enter_context(tc.tile_pool(name="rots", bufs=2))
        expp = ctx.enter_context(tc.tile_pool(name="expp", bufs=3))
        outs = ctx.enter_context(tc.tile_pool(name="outs", bufs=2))
        smalls = ctx.enter_context(tc.tile_pool(name="smalls", bufs=4))
        mmp = ctx.enter_context(tc.tile_pool(name="mmp", bufs=2, space="PSUM"))
        ctxp = ctx.enter_context(tc.tile_pool(name="ctxp", bufs=1, space="PSUM"))
        rsp = ctx.enter_context(tc.tile_pool(name="rsp", bufs=1, space="PSUM"))

        # ---- prewarm the ScalarE exp table set during the initial DMA wait
        warm_in = smalls.tile([1, 1], F32, name="warm_in")
        warm_out = smalls.tile([1, 1], F32, name="warm_out")
        nc.vector.memset(warm_in[:], 0.0)
        nc.scalar.activation(warm_out[:], warm_in[:], AF.Exp)

        def load_w(name, w_d):
            w_sb = const.tile([P, 8 * P], BF, name=name)
            nc.sync.dma_start(
                w_sb.rearrange("p (a m) -> p a m", a=8),
                w_d.rearrange("(a p) m -> p a m", p=P),
            )
            return w_sb

        def load_c(name, t_d):
            """Load the 64 unique table rows, duplicate to partitions 64:128."""
            t_sb = const.tile([P, L], BF, name=name)
            nc.sync.dma_start(t_sb[0:DK, :], t_d[:])
            nc.scalar.dma_start(t_sb[DK:P, :], t_sb[0:DK, :])
            return t_sb

        def load_b(name, b_d):
            b_sb = const.tile([P, 1], F32, name=name)
            nc.sync.dma_start(b_sb[:], b_d[:])
            return b_sb

        ones2 = const.tile([P, 4], BF)
        nc.sync.dma_start(ones2[:], ones2_d[:])
        sel_sb = const.tile([2, P], BF)
        nc.sync.dma_start(sel_sb[:], sel_d[:])
        wq_sb = load_w("wq_sb", wqT)
        bq_sb = load_b("bq_sb", bq_d)

        # persistent activations
        qq_sb = persist.tile([P, TOK], BF)   # roped q-heads  [128 dims, 4096 tok]
        kk_sb = persist.tile([P, TOK], BF)   # roped k-heads
        ctx_sb = persist.tile([P, TOK], BF)  # normalized attention ctx
        # Skew-packed block-diagonal attention operands; chunk c covers 64
        # tokens ktA = 64c..64c+63 per head:
        #   even chunk 2t: [[kh_h0[tokA], 0], [0, kh_h1[tokB]]]
        #   odd chunk 2t+1: [[0, kh_h0[tokB]], [kh_h1[tokA], 0]]  (columns)
        # where tokA/tokB are the halves of token tile t.  The skew keeps
        # every producer->operand copy partition-aligned.
        kh2 = [persist.tile([P, 32 * P], BF, name=f"kh2_{b}") for b in range(B)]
        vh2 = [persist.tile([P, 32 * P], BF, name=f"vh2_{b}") for b in range(B)]
        for t in kh2 + vh2:
            nc.gpsimd.memset(t[:], 0.0)

        # ---- PE warm-up: open the HAM clock gate while input DMA streams
        dum = mmp.tile([P, 512], F32, name="dum", tag="mm")
        for _ in range(24):
            nc.tensor.matmul(dum[:], lhsT=wq_sb[:, 0:P], rhs=wq_sb[:, 0:512],
                             start=True, stop=True, skip_group_check=True)

        # ---------- phase helpers ----------
        def load_x(x_d, g):
            """Queue the 8 input stage tiles for token half g (sync queue)."""
            xts = []
            for kt in range(8):
                xt = stage.tile([P, L], BF, name="xstage", tag="stage")
                nc.sync.dma_start(
                    xt[:], x_d[kt * P:(kt + 1) * P, g * L:(g + 1) * L])
                xts.append(xt)
            return xts

        def proj_compute(xts, w_sb, bias_sb, g, dst_sb, cos_sb, sin_sb):
            """Project token half g (2048 tokens) and apply RoPE (q/k path)."""
            ps = [mmp.tile([P, 1024], F32, name=f"pj{g}_{half}", tag="mm")
                  for half in range(2)]
            for kt in range(8):
                xt = xts[kt]
                for half in range(2):
                    for nb in range(2):
                        c0 = half * 1024 + nb * 512
                        nc.tensor.matmul(
                            ps[half][:, nb * 512:(nb + 1) * 512],
                            lhsT=w_sb[:, kt * P:(kt + 1) * P],
                            rhs=xt[:, c0:c0 + 512],
                            start=(kt == 0), stop=(kt == 7),
                        )
            raw = raws.tile([P, L], BF, name="raw", tag="raw")
            for half in range(2):
                nc.scalar.activation(
                    raw[:, half * 1024:(half + 1) * 1024],
                    ps[half][:], AF.Identity, bias=bias_sb[:])
            rot = rots.tile([P, L], BF, name="rot", tag="rot")
            # rotate-half as partition-block moves (sign folded into sin
            # table); scalar-queue DGE so the sync queue stays free for
            # input prefetch
            for h in range(HPC):
                r0 = h * DK
                nc.scalar.dma_start(rot[r0:r0 + 32, :], raw[r0 + 32:r0 + 64, :])
                nc.scalar.dma_start(rot[r0 + 32:r0 + 64, :], raw[r0:r0 + 32, :])
            dst = dst_sb[:, g * L:(g + 1) * L]
            nc.vector.tensor_mul(raw[:], raw[:], cos_sb[:])
            nc.vector.tensor_mul(rot[:], rot[:], sin_sb[:])
            nc.vector.tensor_add(dst, raw[:], rot[:])

        def proj_v(xts, g):
            """Token-major v projection straight into vh2[g].

            Per 128-token tile t the PSUM holds [tok, vdim]; the skewed
            diagonal blocks are then partition-aligned sub-copies:
              even chunk 2t:  [0:64, 0:64] = h0 tokA, [64:128, 64:128] = h1 tokB
              odd chunk 2t+1: [0:64, 64:128] = h1 tokA, [64:128, 0:64] = h0 tokB
            (bv is folded into the host-side output bias via Wo @ bv.)
            """
            vh2_b = vh2[g]
            for half in range(2):
                vps = mmp.tile([P, 1024], F32, name="vps", tag="mm")
                for t8 in range(8):
                    t = half * 8 + t8
                    for kt in range(8):
                        nc.tensor.matmul(
                            vps[:, t8 * P:(t8 + 1) * P],
                            lhsT=xts[kt][:, t * P:(t + 1) * P],
                            rhs=wv_sb[:, kt * P:(kt + 1) * P],
                            start=(kt == 0), stop=(kt == 7),
                        )
                # strided casts: windows of 256 elements = one chunk pair
                w0 = half * 8 * 256
                dst = vh2_b[0:DK, w0:w0 + 8 * 256].rearrange(
                    "p (t a u) -> p t a u", t=8, u=DK)
                src = vps[0:DK, :].rearrange("p (t a u) -> p t a u", t=8, u=DK)
                # lower partitions: even-col0 (off 0) and odd-col64 (off 192)
                nc.vector.tensor_copy(dst[:, :, 0::3, :], src[:])
                dstu = vh2_b[DK:P, w0:w0 + 8 * 256].rearrange(
                    "p (t a u) -> p t a u", t=8, u=DK)
                srcu = vps[DK:P, :].rearrange("p (t a u) -> p t a u", t=8, u=DK)
                # upper partitions: even-col64 (off 64) <- h1 cols,
                #                   odd-col0 (off 128) <- h0 cols
                nc.vector.tensor_copy(dstu[:, :, 1:2, :], srcu[:, :, 1:2, :])
                nc.vector.tensor_copy(dstu[:, :, 2:3, :], srcu[:, :, 0:1, :])

        def build_kh2(b):
            """Fill kh2[b] from roped kk (4 partition-aligned strided copies)."""
            kh2_r = kh2[b].rearrange("p (t e u) -> p t e u", t=16, u=P)
            kk_r = kk_sb[:, b * L:(b + 1) * L].rearrange(
                "p (t s u) -> p t s u", t=16, u=DK)
            nc.vector.tensor_copy(kh2_r[0:DK, :, 0:1, 0:DK], kk_r[0:DK, :, 0:1, :])
            nc.vector.tensor_copy(kh2_r[DK:P, :, 0:1, DK:P], kk_r[DK:P, :, 1:2, :])
            nc.vector.tensor_copy(kh2_r[DK:P, :, 1:2, 0:DK], kk_r[DK:P, :, 0:1, :])
            nc.vector.tensor_copy(kh2_r[0:DK, :, 1:2, DK:P], kk_r[0:DK, :, 1:2, :])

        def attention(b, q2):
            """Both heads at once via skew-packed block-diagonal K=128 matmuls."""
            qs = qq_sb[:, b * L + q2 * 1024: b * L + (q2 + 1) * 1024]
            cp = ctxp.tile([P, 1024], F32, name="cp", tag="ctx")
            rs = rsp.tile([2, 1024], F32, name="rs", tag="rs")
            ex_prev = None
            for c in range(33):
                ex_cur = None
                if c < 32:
                    sc = mmp.tile([P, 1024], F32, name="sc", tag="mm")
                    for nb in range(2):
                        nc.tensor.matmul(
                            sc[:, nb * 512:(nb + 1) * 512],
                            lhsT=kh2[b][:, c * P:(c + 1) * P],
                            rhs=qs[:, nb * 512:(nb + 1) * 512],
                            start=True, stop=True, skip_group_check=True,
                        )
                    ex_cur = expp.tile([P, 1024], BF, name="ex", tag="exp")
                    nc.scalar.activation(ex_cur[:], sc[:], AF.Exp)
                if c >= 1:
                    cpv = c - 1
                    for nb in range(2):
                        sl = slice(nb * 512, (nb + 1) * 512)
                        nc.tensor.matmul(
                            cp[:, sl], lhsT=vh2[b][:, cpv * P:(cpv + 1) * P],
                            rhs=ex_prev[:, sl],
                            start=(cpv == 0), stop=(cpv == 31),
                            skip_group_check=True,
                        )
                    o2 = ones2[:, 0:2] if cpv % 2 == 0 else ones2[:, 2:4]
                    for nb in range(2):
                        sl = slice(nb * 512, (nb + 1) * 512)
                        nc.tensor.matmul(
                            rs[:, sl], lhsT=o2, rhs=ex_prev[:, sl],
                            start=(cpv == 0), stop=(cpv == 31),
                            skip_group_check=True,
                        )
                ex_prev = ex_cur
            # normalize: reciprocal of the two denominator rows, broadcast
            # across partitions with a K=2 matmul, one DVE multiply.
            rsum = smalls.tile([2, 1024], F32, name="rsum", tag="rsum", bufs=2)
            nc.vector.tensor_copy(rsum[:], rs[:])
            rec = smalls.tile([2, 1024], F32, name="rec", tag="rec", bufs=2)
            nc.vector.reciprocal_approx_fast(rec[:], rsum[:])
            rec_bf = smalls.tile([2, 1024], BF, name="recbf", tag="recbf", bufs=2)
            nc.vector.tensor_copy(rec_bf[:], rec[:])
            bcs = rsp.tile([P, 1024], F32, name="bcs", tag="rs")
            for nb in range(2):
                sl = slice(nb * 512, (nb + 1) * 512)
                nc.tensor.matmul(
                    bcs[:, sl], lhsT=sel_sb[:], rhs=rec_bf[:, sl],
                    start=True, stop=True, skip_group_check=True,
                )
            craw = smalls.tile([P, 1024], BF, name="craw", tag="craw", bufs=2)
            nc.vector.tensor_copy(craw[:], cp[:])
            c0 = b * L + q2 * 1024
            nc.vector.tensor_mul(ctx_sb[:, c0:c0 + 1024], craw[:], bcs[:])

        def out_proj(b, q2):
            ob = None
            for i, tb in enumerate(range(q2 * 8, q2 * 8 + 8)):
                t0 = b * L + tb * P
                if i % 4 == 0:
                    ob = outs.tile([P, 4 * D], BF, name="ob", tag="out")
                po = mmp.tile([P, D], F32, name="po", tag="mm")
                for nb in range(2):
                    nc.tensor.matmul(
                        po[:, nb * 512:(nb + 1) * 512],
                        lhsT=ctx_sb[:, t0:t0 + P],
                        rhs=wo_sb[:, nb * 512:(nb + 1) * 512],
                        start=True, stop=True, skip_group_check=True,
                    )
                # evictions: batch 0 overlaps the rope-heavy transition
                # (DVE busy -> use ScalarE); batch 1 overlaps attention
                # (ScalarE exp-saturated -> use DVE)
                dst = ob[:, (i % 4) * D:(i % 4 + 1) * D]
                if b == 0:
                    nc.scalar.activation(dst, po[:], AF.Identity)
                else:
                    nc.vector.tensor_copy(dst, po[:])
                if i % 4 == 3:
                    t00 = b * L + (tb - 3) * P
                    nc.sync.dma_start(
                        outp[t00:t00 + 4 * P, :].rearrange(
                            "(a p) d -> p a d", p=P),
                        ob.rearrange("p (a d) -> p a d", a=4))

        # ---------- program ----------
        xq0 = load_x(qT, 0)
        cq_sb = load_c("cq_sb", cos_q)
        sq_sb = load_c("sq_sb", sin_q)
        proj_compute(xq0, wq_sb, bq_sb, 0, qq_sb, cq_sb, sq_sb)
        wk_sb = load_w("wk_sb", wkT)
        bk_sb = load_b("bk_sb", bk_d)
        xk0 = load_x(kT, 0)
        ck_sb = load_c("ck_sb", cos_k)
        sk_sb = load_c("sk_sb", sin_k)
        proj_compute(xk0, wk_sb, bk_sb, 0, kk_sb, ck_sb, sk_sb)
        build_kh2(0)
        wv_sb = load_w("wv_sb", wvT)
        wo_sb = const.tile([P, D], BF)
        nc.sync.dma_start(wo_sb[:], woT[:])
        xv0 = load_x(vT, 0)
        proj_v(xv0, 0)
        # queue batch-1 input DMA now: it streams on the sync queue while
        # the PE runs batch-0 attention; batch-1 projections are slotted
        # between attention phases so stage buffers keep recycling
        xq1 = load_x(qT, 1)
        xk1 = load_x(kT, 1)
        xv1 = load_x(vT, 1)
        attention(0, 0)
        proj_compute(xq1, wq_sb, bq_sb, 1, qq_sb, cq_sb, sq_sb)
        attention(0, 1)
        proj_compute(xk1, wk_sb, bk_sb, 1, kk_sb, ck_sb, sk_sb)
        out_proj(0, 0)
        out_proj(0, 1)
        proj_v(xv1, 1)
        build_kh2(1)
        attention(1, 0)
        attention(1, 1)
        out_proj(1, 0)
        out_proj(1, 1)

    return nc


def _rope_tables():
    """Host-built RoPE tables, transposed to [d, t] (unique 64 rows).

    sin is sign-folded for the rotate-half convention; q tables carry the
    1/sqrt(dk) attention scale.  The kernel duplicates rows onto
    partitions 64:128 on-chip.
    """
    inv_freq = 1.0 / (ROPE_BASE ** (np.arange(0, DK, 2, dtype=np.float64) / DK))
    t = np.arange(L, dtype=np.float64)
    ang = np.outer(t, inv_freq)               # [L, 32]
    emb = np.concatenate([ang, ang], axis=1)  # [L, 64]
    cos = np.cos(emb).T.astype(np.float32)    # [64, L]
    sin = np.sin(emb).T.astype(np.float32)
    sin_folded = sin.copy()
    sin_folded[:32] *= -1.0
    scale = 1.0 / np.sqrt(DK)
    bf = ml_dtypes.bfloat16
    return (
        (cos * scale).astype(bf), (sin_folded * scale).astype(bf),
        cos.astype(bf), sin_folded.astype(bf),
    )


_NC_CACHE = {}


def _get_nc():
    if "nc" not in _NC_CACHE:
        nc = build_nc()
        nc.finalize()
        _NC_CACHE["nc"] = nc
    return _NC_CACHE["nc"]


def _in_maps(q, k, v, Wq, bq, Wk, bk, Wv, Wo):
    bf = ml_dtypes.bfloat16
    qT = np.ascontiguousarray(q.reshape(TOK, D).T).astype(bf)
    kT = np.ascontiguousarray(k.reshape(TOK, D).T).astype(bf)
    vT = np.ascontiguousarray(v.reshape(TOK, D).T).astype(bf)
    cos_q, sin_q, cos_k, sin_k = _rope_tables()
    # even-chunk selectors: rows 0:64 = h0, rows 64:128 = h1
    # odd-chunk selectors (skewed): rows 0:64 = h1, rows 64:128 = h0
    ones2 = np.zeros((P, 4), bf)
    ones2[0:DK, 0] = 1
    ones2[DK:P, 1] = 1
    ones2[DK:P, 2] = 1
    ones2[0:DK, 3] = 1
    sel = np.zeros((2, P), bf)
    sel[0, 0:DK] = 1
    sel[1, DK:P] = 1
    in_maps = []
    for c in range(NCORES):
        hs = slice(c * PD, (c + 1) * PD)
        in_maps.append({
            "qT": qT, "kT": kT, "vT": vT,
            "wqT": np.ascontiguousarray(Wq[hs, :].T).astype(bf),
            "wkT": np.ascontiguousarray(Wk[hs, :].T).astype(bf),
            "wvT": np.ascontiguousarray(Wv[hs, :].T).astype(bf),
            "woT": np.ascontiguousarray(Wo[:, hs].T).astype(bf),
            "bq": np.asarray(bq[hs], np.float32).reshape(PD, 1),
            "bk": np.asarray(bk[hs], np.float32).reshape(PD, 1),
            "cos_q": cos_q, "sin_q": sin_q, "cos_k": cos_k, "sin_k": sin_k,
            "ones2": ones2, "sel": sel,
        })
    return in_maps


def kernel(q, k, v, Wq, bq, Wk, bk, Wv, bv, Wo, bo):
    assert q.shape == (B, L, D) and k.shape == (B, L, D) and v.shape == (B, L, D)
    in_maps = _in_maps(q, k, v, Wq, bq, Wk, bk, Wv, Wo)
    nc = _get_nc()
    res = run_bass_kernel_spmd(nc, in_maps, list(range(NCORES)))
    out = np.zeros((TOK, D), np.float64)
    for r in res.results:
        out += r["outp"].astype(np.float64)
    # bv never touches the device: since the attention weights sum to 1,
    # its contribution is the constant vector Wo @ bv
    out += np.asarray(bo, np.float64)[None, :]
    out += (np.asarray(Wo, np.float64) @ np.asarray(bv, np.float64))[None, :]
    return out.astype(np.float32).reshape(B, L, D)


# revision 20
# speedup vs baseline: 1.0936x; 1.0023x over previous
"""Trainium2 Bass kernel for MultiHeadAttention with RoPE.

Problem: B=2, L=2048, d_model=1024, 16 heads, d_k=64, fp32 in/out.

Sharding (8 cores): tensor-parallel over heads — core c owns heads
{2c, 2c+1}, i.e. a 128-wide slice of the projection output dims.  Every
core reads the full q/k/v activations (transposed + bf16 on host), its
own 128-row slice of Wq/Wk/Wv (pre-transposed) and the matching 128
columns of Wo.  Each core computes its heads' attention output and the
partial d_model-sized output projection; the host sums the 8 partials
and adds bo (+ Wo @ bv, folded out of the device program).

Per-core pipeline (all matmuls bf16, fp32 PSUM accumulation):
  1. q/k projections:  qh.T = WqT.T @ q.T  laid out [128 head-dims, 2048 tok]
     per batch, RoPE applied via partition-shifted DMA copy + 3 DVE ops
     (1/sqrt(dk) scale and rotate-half sign folded into host tables)
  2. v projection runs token-major (stationary = x tile, moving = WvT
     chunk) so its PSUM tiles are already [token, dim] and evict straight
     into the block-diagonal vh2 operand with plain DVE casts — no PE
     transposes, no staging buffer, no serial placement phase
  3. scores.T chunks [kt-pack 128, qt 512] = kh2 (block-diag stationary,
     K=128 so the HAM clock-gate opens to 2.4 GHz) @ qh
  4. exp on ScalarE (no max-subtract: scores ~ N(0,1)), bf16 out
  5. ctx accumulation via block-diag vh2 stationary; parallel [128, 2]
     ones matmuls accumulate both heads' softmax denominators in [2, qt]
     PSUM.  The diagonal packing is *skewed* (odd chunks anti-diagonal)
     so the token-major v eviction never crosses partitions; the ones
     stationary alternates parity to keep head sums separate.
  6. normalize: DVE reciprocal of the denominators, K=2 PE matmul
     broadcast across partitions, one DVE multiply (flash-style deferred
     normalization)
  7. out_partial[tok, 1024] = ctx (stationary) @ WoT slice; evictions on
     ScalarE (DVE is busy with rope), stores batched 4 tiles per DMA

Scheduling: bulk loads/stores ride the sync DMA queue; compute-dependent
moves (rope rotate, table duplication) use the scalar-engine hardware DGE
queue.  Batch-1 input DMA is queued before batch-0 attention and batch-1
projections are interleaved *between* attention phases so the 10-buffer
stage pool keeps recycling and the DMA never goes idle.  Warm-up matmuls
at t~2us open the HAM clock gate before the first projection.
"""

import os
import numpy as np
import ml_dtypes

import concourse.bass as bass
import concourse.mybir as mybir
import concourse.tile as tile
from concourse import bacc
from concourse.bass_utils import run_bass_kernel_spmd

BF = mybir.dt.bfloat16
F32 = mybir.dt.float32
AF = mybir.ActivationFunctionType

NCORES = 8
B = 2
L = 2048
D = 1024          # d_model
H = 16            # heads
DK = 64           # head dim
HPC = H // NCORES  # heads per core = 2
PD = HPC * DK      # projection dims per core = 128
TOK = B * L        # 4096 tokens
P = 128

ROPE_BASE = 10000.0


def build_nc(debug_dumps=False):
    """Build the single-core Bass program (SPMD: same program, per-core data)."""
    from contextlib import ExitStack

    nc = bacc.Bacc("TRN2", target_bir_lowering=False, debug=False)

    # ---- DRAM I/O ----
    qT = nc.dram_tensor("qT", [D, TOK], BF, kind="ExternalInput").ap()
    kT = nc.dram_tensor("kT", [D, TOK], BF, kind="ExternalInput").ap()
    vT = nc.dram_tensor("vT", [D, TOK], BF, kind="ExternalInput").ap()
    wqT = nc.dram_tensor("wqT", [D, PD], BF, kind="ExternalInput").ap()
    wkT = nc.dram_tensor("wkT", [D, PD], BF, kind="ExternalInput").ap()
    wvT = nc.dram_tensor("wvT", [D, PD], BF, kind="ExternalInput").ap()
    woT = nc.dram_tensor("woT", [PD, D], BF, kind="ExternalInput").ap()
    bq_d = nc.dram_tensor("bq", [PD, 1], F32, kind="ExternalInput").ap()
    bk_d = nc.dram_tensor("bk", [PD, 1], F32, kind="ExternalInput").ap()
    # RoPE tables ship as the unique 64 head-dim rows; duplicated on-chip
    cos_q = nc.dram_tensor("cos_q", [DK, L], BF, kind="ExternalInput").ap()
    sin_q = nc.dram_tensor("sin_q", [DK, L], BF, kind="ExternalInput").ap()
    cos_k = nc.dram_tensor("cos_k", [DK, L], BF, kind="ExternalInput").ap()
    sin_k = nc.dram_tensor("sin_k", [DK, L], BF, kind="ExternalInput").ap()
    # ones2 [128, 4]: cols 0/1 = even-chunk head-0/head-1 row selectors,
    # cols 2/3 = odd-chunk selectors (skewed packing swaps the halves)
    ones2_d = nc.dram_tensor("ones2", [P, 4], BF, kind="ExternalInput").ap()
    sel_d = nc.dram_tensor("sel", [2, P], BF, kind="ExternalInput").ap()
    outp = nc.dram_tensor("outp", [TOK, D], BF, kind="ExternalOutput").ap()

    with tile.TileContext(nc) as tc, ExitStack() as ctx:
        const = ctx.enter_context(tc.tile_pool(name="const", bufs=1))
        persist = ctx.enter_context(tc.tile_pool(name="persist", bufs=1))
        stage = ctx.enter_context(tc.tile_pool(name="stage", bufs=10))
        raws = ctx.enter_context(tc.tile_pool(name="raws", bufs=2))
        rots = ctx.enter_context(tc.tile_pool(name="rots", bufs=2))
        expp = ctx.enter_context(tc.tile_pool(name="expp", bufs=3))
        outs = ctx.enter_context(tc.tile_pool(name="outs", bufs=2))
        smalls = ctx.enter_context(tc.tile_pool(name="smalls", bufs=4))
        mmp = ctx.enter_context(tc.tile_pool(name="mmp", bufs=2, space="PSUM"))
        ctxp = ctx.enter_context(tc.tile_pool(name="ctxp", bufs=1, space="PSUM"))
        rsp = ctx.enter_context(tc.tile_pool(name="rsp", bufs=1, space="PSUM"))

        # ---- prewarm the ScalarE exp table set during the initial DMA wait
        warm_in = smalls.tile([1, 1], F32, name="warm_in")
        warm_out = smalls.tile([1, 1], F32, name="warm_out")
        nc.vector.memset(warm_in[:], 0.0)
        nc.scalar.activation(warm_out[:], warm_in[:], AF.Exp)

        def load_w(name, w_d):
            w_sb = const.tile([P, 8 * P], BF, name=name)
            nc.sync.dma_start(
                w_sb.rearrange("p (a m) -> p a m", a=8),
                w_d.rearrange("(a p) m -> p a m", p=P),
            )
            return w_sb

        def load_c(name, t_d):
            """Load the 64 unique table rows, duplicate to partitions 64:128."""
            t_sb = const.tile([P, L], BF, name=name)
            nc.sync.dma_start(t_sb[0:DK, :], t_d[:])
            nc.scalar.dma_start(t_sb[DK:P, :], t_sb[0:DK, :])
            return t_sb

        def load_b(name, b_d):
            b_sb = const.tile([P, 1], F32, name=name)
            nc.sync.dma_start(b_sb[:], b_d[:])
            return b_sb

        ones2 = const.tile([P, 4], BF)
        nc.sync.dma_start(ones2[:], ones2_d[:])
        sel_sb = const.tile([2, P], BF)
        nc.sync.dma_start(sel_sb[:], sel_d[:])
        wq_sb = load_w("wq_sb", wqT)
        bq_sb = load_b("bq_sb", bq_d)

        # persistent activations
        qq_sb = persist.tile([P, TOK], BF)   # roped q-heads  [128 dims, 4096 tok]
        kk_sb = persist.tile([P, TOK], BF)   # roped k-heads
        ctx_sb = persist.tile([P, TOK], BF)  # normalized attention ctx
        # Skew-packed block-diagonal attention operands; chunk c covers 64
        # tokens ktA = 64c..64c+63 per head:
        #   even chunk 2t: [[kh_h0[tokA], 0], [0, kh_h1[tokB]]]
        #   odd chunk 2t+1: [[0, kh_h0[tokB]], [kh_h1[tokA], 0]]  (columns)
        # where tokA/tokB are the halves of token tile t.  The skew keeps
        # every producer->operand copy partition-aligned.
        kh2 = [persist.tile([P, 32 * P], BF, name=f"kh2_{b}") for b in range(B)]
        vh2 = [persist.tile([P, 32 * P], BF, name=f"vh2_{b}") for b in range(B)]
        for t in kh2 + vh2:
            nc.gpsimd.memset(t[:], 0.0)

        # ---- PE warm-up: open the HAM clock gate while input DMA streams.
        # Dummies use the (phase-A-idle) rsp PSUM bank so they never
        # contend with the projection tiles in mmp.
        def dummies(n):
            dmt = rsp.tile([P, 512], F32, name="dum", tag="rs")
            for _ in range(n):
                nc.tensor.matmul(dmt[:], lhsT=wq_sb[:, 0:P],
                                 rhs=wq_sb[:, 0:512],
                                 start=True, stop=True, skip_group_check=True)

        dummies(24)

        # ---------- phase helpers ----------
        def load_x(x_d, g):
            """Queue the 8 input stage tiles for token half g (sync queue)."""
            xts = []
            for kt in range(8):
                xt = stage.tile([P, L], BF, name="xstage", tag="stage")
                nc.sync.dma_start(
                    xt[:], x_d[kt * P:(kt + 1) * P, g * L:(g + 1) * L])
                xts.append(xt)
            return xts

        def proj_compute(xts, w_sb, bias_sb, g, dst_sb, cos_sb, sin_sb):
            """Project token half g (2048 tokens) and apply RoPE (q/k path)."""
            ps = [mmp.tile([P, 1024], F32, name=f"pj{g}_{half}", tag="mm")
                  for half in range(2)]
            for kt in range(8):
                xt = xts[kt]
                for half in range(2):
                    for nb in range(2):
                        c0 = half * 1024 + nb * 512
                        nc.tensor.matmul(
                            ps[half][:, nb * 512:(nb + 1) * 512],
                            lhsT=w_sb[:, kt * P:(kt + 1) * P],
                            rhs=xt[:, c0:c0 + 512],
                            start=(kt == 0), stop=(kt == 7),
                        )
            raw = raws.tile([P, L], BF, name="raw", tag="raw")
            for half in range(2):
                nc.scalar.activation(
                    raw[:, half * 1024:(half + 1) * 1024],
                    ps[half][:], AF.Identity, bias=bias_sb[:])
            rot = rots.tile([P, L], BF, name="rot", tag="rot")
            # rotate-half as partition-block moves (sign folded into sin
            # table); scalar-queue DGE so the sync queue stays free for
            # input prefetch
            for h in range(HPC):
                r0 = h * DK
                nc.scalar.dma_start(rot[r0:r0 + 32, :], raw[r0 + 32:r0 + 64, :])
                nc.scalar.dma_start(rot[r0 + 32:r0 + 64, :], raw[r0:r0 + 32, :])
            dst = dst_sb[:, g * L:(g + 1) * L]
            nc.vector.tensor_mul(raw[:], raw[:], cos_sb[:])
            nc.vector.tensor_mul(rot[:], rot[:], sin_sb[:])
            nc.vector.tensor_add(dst, raw[:], rot[:])

        def proj_v(xts, g):
            """Token-major v projection straight into vh2[g].

            Per 128-token tile t the PSUM holds [tok, vdim]; the skewed
            diagonal blocks are then partition-aligned sub-copies:
              even chunk 2t:  [0:64, 0:64] = h0 tokA, [64:128, 64:128] = h1 tokB
              odd chunk 2t+1: [0:64, 64:128] = h1 tokA, [64:128, 0:64] = h0 tokB
            (bv is folded into the host-side output bias via Wo @ bv.)
            """
            vh2_b = vh2[g]
            for half in range(2):
                vps = mmp.tile([P, 1024], F32, name="vps", tag="mm")
                for t8 in range(8):
                    t = half * 8 + t8
                    for kt in range(8):
                        nc.tensor.matmul(
                            vps[:, t8 * P:(t8 + 1) * P],
                            lhsT=xts[kt][:, t * P:(t + 1) * P],
                            rhs=wv_sb[:, kt * P:(kt + 1) * P],
                            start=(kt == 0), stop=(kt == 7),
                        )
                # strided casts: windows of 256 elements = one chunk pair
                w0 = half * 8 * 256
                dst = vh2_b[0:DK, w0:w0 + 8 * 256].rearrange(
                    "p (t a u) -> p t a u", t=8, u=DK)
                src = vps[0:DK, :].rearrange("p (t a u) -> p t a u", t=8, u=DK)
                # lower partitions: even-col0 (off 0) and odd-col64 (off 192)
                nc.vector.tensor_copy(dst[:, :, 0::3, :], src[:])
                dstu = vh2_b[DK:P, w0:w0 + 8 * 256].rearrange(
                    "p (t a u) -> p t a u", t=8, u=DK)
                srcu = vps[DK:P, :].rearrange("p (t a u) -> p t a u", t=8, u=DK)
                # upper partitions: even-col64 (off 64) <- h1 cols,
                #                   odd-col0 (off 128) <- h0 cols
                nc.vector.tensor_copy(dstu[:, :, 1:2, :], srcu[:, :, 1:2, :])
                nc.vector.tensor_copy(dstu[:, :, 2:3, :], srcu[:, :, 0:1, :])

        def build_kh2(b):
            """Fill kh2[b] from roped kk (4 partition-aligned strided copies)."""
            kh2_r = kh2[b].rearrange("p (t e u) -> p t e u", t=16, u=P)
            kk_r = kk_sb[:, b * L:(b + 1) * L].rearrange(
                "p (t s u) -> p t s u", t=16, u=DK)
            nc.vector.tensor_copy(kh2_r[0:DK, :, 0:1, 0:DK], kk_r[0:DK, :, 0:1, :])
            nc.vector.tensor_copy(kh2_r[DK:P, :, 0:1, DK:P], kk_r[DK:P, :, 1:2, :])
            nc.vector.tensor_copy(kh2_r[DK:P, :, 1:2, 0:DK], kk_r[DK:P, :, 0:1, :])
            nc.vector.tensor_copy(kh2_r[0:DK, :, 1:2, DK:P], kk_r[0:DK, :, 1:2, :])

        def attention(b, q2):
            """Both heads at once via skew-packed block-diagonal K=128 matmuls."""
            qs = qq_sb[:, b * L + q2 * 1024: b * L + (q2 + 1) * 1024]
            cp = ctxp.tile([P, 1024], F32, name="cp", tag="ctx")
            rs = rsp.tile([2, 1024], F32, name="rs", tag="rs")
            ex_prev = None
            for c in range(33):
                ex_cur = None
                if c < 32:
                    sc = mmp.tile([P, 1024], F32, name="sc", tag="mm")
                    for nb in range(2):
                        nc.tensor.matmul(
                            sc[:, nb * 512:(nb + 1) * 512],
                            lhsT=kh2[b][:, c * P:(c + 1) * P],
                            rhs=qs[:, nb * 512:(nb + 1) * 512],
                            start=True, stop=True, skip_group_check=True,
                        )
                    ex_cur = expp.tile([P, 1024], BF, name="ex", tag="exp")
                    nc.scalar.activation(ex_cur[:], sc[:], AF.Exp)
                if c >= 1:
                    cpv = c - 1
                    for nb in range(2):
                        sl = slice(nb * 512, (nb + 1) * 512)
                        nc.tensor.matmul(
                            cp[:, sl], lhsT=vh2[b][:, cpv * P:(cpv + 1) * P],
                            rhs=ex_prev[:, sl],
                            start=(cpv == 0), stop=(cpv == 31),
                            skip_group_check=True,
                        )
                    o2 = ones2[:, 0:2] if cpv % 2 == 0 else ones2[:, 2:4]
                    for nb in range(2):
                        sl = slice(nb * 512, (nb + 1) * 512)
                        nc.tensor.matmul(
                            rs[:, sl], lhsT=o2, rhs=ex_prev[:, sl],
                            start=(cpv == 0), stop=(cpv == 31),
                            skip_group_check=True,
                        )
                ex_prev = ex_cur
            # normalize: reciprocal of the two denominator rows, broadcast
            # across partitions with a K=2 matmul, one DVE multiply.
            rsum = smalls.tile([2, 1024], F32, name="rsum", tag="rsum", bufs=2)
            nc.vector.tensor_copy(rsum[:], rs[:])
            rec = smalls.tile([2, 1024], F32, name="rec", tag="rec", bufs=2)
            nc.vector.reciprocal_approx_fast(rec[:], rsum[:])
            rec_bf = smalls.tile([2, 1024], BF, name="recbf", tag="recbf", bufs=2)
            nc.vector.tensor_copy(rec_bf[:], rec[:])
            bcs = rsp.tile([P, 1024], F32, name="bcs", tag="rs")
            for nb in range(2):
                sl = slice(nb * 512, (nb + 1) * 512)
                nc.tensor.matmul(
                    bcs[:, sl], lhsT=sel_sb[:], rhs=rec_bf[:, sl],
                    start=True, stop=True, skip_group_check=True,
                )
            craw = smalls.tile([P, 1024], BF, name="craw", tag="craw", bufs=2)
            # ScalarE evicts cp in parallel with the DVE reciprocal chain
            nc.scalar.activation(craw[:], cp[:], AF.Identity)
            c0 = b * L + q2 * 1024
            nc.vector.tensor_mul(ctx_sb[:, c0:c0 + 1024], craw[:], bcs[:])

        def out_proj(b, q2):
            ob = None
            for i, tb in enumerate(range(q2 * 8, q2 * 8 + 8)):
                t0 = b * L + tb * P
                if i % 4 == 0:
                    ob = outs.tile([P, 4 * D], BF, name="ob", tag="out")
                po = mmp.tile([P, D], F32, name="po", tag="mm")
                for nb in range(2):
                    nc.tensor.matmul(
                        po[:, nb * 512:(nb + 1) * 512],
                        lhsT=ctx_sb[:, t0:t0 + P],
                        rhs=wo_sb[:, nb * 512:(nb + 1) * 512],
                        start=True, stop=True, skip_group_check=True,
                    )
                # evict on ScalarE: out_proj never overlaps attention exps
                # in this schedule, and DVE is the normalize/rope engine
                nc.scalar.activation(
                    ob[:, (i % 4) * D:(i % 4 + 1) * D], po[:], AF.Identity)
                if i % 4 == 3:
                    t00 = b * L + (tb - 3) * P
                    nc.sync.dma_start(
                        outp[t00:t00 + 4 * P, :].rearrange(
                            "(a p) d -> p a d", p=P),
                        ob.rearrange("p (a d) -> p a d", a=4))

        # ---------- program ----------
        # phase A order v -> q -> k: the LDW-bound v projection trickles
        # under the q/k loads, and the rope tails land just before
        # attention instead of serializing after it
        wv_sb = load_w("wv_sb", wvT)
        xv0 = load_x(vT, 0)
        proj_v(xv0, 0)
        dummies(10)
        cq_sb = load_c("cq_sb", cos_q)
        sq_sb = load_c("sq_sb", sin_q)
        xq0 = load_x(qT, 0)
        proj_compute(xq0, wq_sb, bq_sb, 0, qq_sb, cq_sb, sq_sb)
        dummies(10)
        wk_sb = load_w("wk_sb", wkT)
        bk_sb = load_b("bk_sb", bk_d)
        xk0 = load_x(kT, 0)
        ck_sb = load_c("ck_sb", cos_k)
        sk_sb = load_c("sk_sb", sin_k)
        proj_compute(xk0, wk_sb, bk_sb, 0, kk_sb, ck_sb, sk_sb)
        build_kh2(0)
        wo_sb = const.tile([P, D], BF)
        nc.sync.dma_start(wo_sb[:], woT[:])
        # queue batch-1 input DMA now: it streams on the sync queue while
        # the PE runs batch-0 attention; batch-1 projections are slotted
        # between attention phases so stage buffers keep recycling
        xv1 = load_x(vT, 1)
        xq1 = load_x(qT, 1)
        xk1 = load_x(kT, 1)
        attention(0, 0)
        proj_v(xv1, 1)
        attention(0, 1)
        proj_compute(xq1, wq_sb, bq_sb, 1, qq_sb, cq_sb, sq_sb)
        out_proj(0, 0)
        proj_compute(xk1, wk_sb, bk_sb, 1, kk_sb, ck_sb, sk_sb)
        build_kh2(1)
        out_proj(0, 1)
        attention(1, 0)
        attention(1, 1)
        out_proj(1, 0)
        out_proj(1, 1)

    return nc


def _rope_tables():
    """Host-built RoPE tables, transposed to [d, t] (unique 64 rows).

    sin is sign-folded for the rotate-half convention; q tables carry the
    1/sqrt(dk) attention scale.  The kernel duplicates rows onto
    partitions 64:128 on-chip.
    """
    inv_freq = 1.0 / (ROPE_BASE ** (np.arange(0, DK, 2, dtype=np.float64) / DK))
    t = np.arange(L, dtype=np.float64)
    ang = np.outer(t, inv_freq)               # [L, 32]
    emb = np.concatenate([ang, ang], axis=1)  # [L, 64]
    cos = np.cos(emb).T.astype(np.float32)    # [64, L]
    sin = np.sin(emb).T.astype(np.float32)
    sin_folded = sin.copy()
    sin_folded[:32] *= -1.0
    scale = 1.0 / np.sqrt(DK)
    bf = ml_dtypes.bfloat16
    return (
        (cos * scale).astype(bf), (sin_folded * scale).astype(bf),
        cos.astype(bf), sin_folded.astype(bf),
    )


_NC_CACHE = {}


def _get_nc():
    if "nc" not in _NC_CACHE:
        nc = build_nc()
        nc.finalize()
        _NC_CACHE["nc"] = nc
    return _NC_CACHE["nc"]


def _in_maps(q, k, v, Wq, bq, Wk, bk, Wv, Wo):
    bf = ml_dtypes.bfloat16
    qT = np.ascontiguousarray(q.reshape(TOK, D).T).astype(bf)
    kT = np.ascontiguousarray(k.reshape(TOK, D).T).astype(bf)
    vT = np.ascontiguousarray(v.reshape(TOK, D).T).astype(bf)
    cos_q, sin_q, cos_k, sin_k = _rope_tables()
    # even-chunk selectors: rows 0:64 = h0, rows 64:128 = h1
    # odd-chunk selectors (skewed): rows 0:64 = h1, rows 64:128 = h0
    ones2 = np.zeros((P, 4), bf)
    ones2[0:DK, 0] = 1
    ones2[DK:P, 1] = 1
    ones2[DK:P, 2] = 1
    ones2[0:DK, 3] = 1
    sel = np.zeros((2, P), bf)
    sel[0, 0:DK] = 1
    sel[1, DK:P] = 1
    in_maps = []
    for c in range(NCORES):
        hs = slice(c * PD, (c + 1) * PD)
        in_maps.append({
            "qT": qT, "kT": kT, "vT": vT,
            "wqT": np.ascontiguousarray(Wq[hs, :].T).astype(bf),
            "wkT": np.ascontiguousarray(Wk[hs, :].T).astype(bf),
            "wvT": np.ascontiguousarray(Wv[hs, :].T).astype(bf),
            "woT": np.ascontiguousarray(Wo[:, hs].T).astype(bf),
            "bq": np.asarray(bq[hs], np.float32).reshape(PD, 1),
            "bk": np.asarray(bk[hs], np.float32).reshape(PD, 1),
            "cos_q": cos_q, "sin_q": sin_q, "cos_k": cos_k, "sin_k": sin_k,
            "ones2": ones2, "sel": sel,
        })
    return in_maps


def kernel(q, k, v, Wq, bq, Wk, bk, Wv, bv, Wo, bo):
    assert q.shape == (B, L, D) and k.shape == (B, L, D) and v.shape == (B, L, D)
    in_maps = _in_maps(q, k, v, Wq, bq, Wk, bk, Wv, Wo)
    nc = _get_nc()
    res = run_bass_kernel_spmd(nc, in_maps, list(range(NCORES)))
    out = np.zeros((TOK, D), np.float64)
    for r in res.results:
        out += r["outp"].astype(np.float64)
    # bv never touches the device: since the attention weights sum to 1,
    # its contribution is the constant vector Wo @ bv
    out += np.asarray(bo, np.float64)[None, :]
    out += (np.asarray(Wo, np.float64) @ np.asarray(bv, np.float64))[None, :]
    return out.astype(np.float32).reshape(B, L, D)


# revision 26
# speedup vs baseline: 1.1281x; 1.0315x over previous
"""Trainium2 Bass kernel for MultiHeadAttention with RoPE.

Problem: B=2, L=2048, d_model=1024, 16 heads, d_k=64, fp32 in/out.

Sharding (8 cores): tensor-parallel over heads — core c owns heads
{2c, 2c+1}, i.e. a 128-wide slice of the projection output dims.  Every
core reads the full q/k/v activations (transposed + bf16 on host), its
own 128-row slice of Wq/Wk/Wv (pre-transposed) and the matching 128
columns of Wo.  Each core computes its heads' attention output and the
partial d_model-sized output projection; the host sums the 8 partials
and adds bo (+ Wo @ bv, folded out of the device program).

Per-core pipeline (all matmuls bf16, fp32 PSUM accumulation):
  1. q/k projections:  qh.T = WqT.T @ q.T  laid out [128 head-dims, 2048 tok]
     per batch, RoPE applied via partition-shifted DMA copy + 3 DVE ops
     (1/sqrt(dk) scale and rotate-half sign folded into host tables)
  2. v projection runs token-major (stationary = x tile, moving = WvT
     chunk) so its PSUM tiles are already [token, dim] and evict straight
     into the block-diagonal vh2 operand with plain DVE casts — no PE
     transposes, no staging buffer, no serial placement phase
  3. scores.T chunks [kt-pack 128, qt 512] = kh2 (block-diag stationary,
     K=128 so the HAM clock-gate opens to 2.4 GHz) @ qh
  4. exp on ScalarE (no max-subtract: scores ~ N(0,1)), bf16 out
  5. ctx accumulation via block-diag vh2 stationary; parallel [128, 2]
     ones matmuls accumulate both heads' softmax denominators in [2, qt]
     PSUM.  The diagonal packing is *skewed* (odd chunks anti-diagonal)
     so the token-major v eviction never crosses partitions; the ones
     stationary alternates parity to keep head sums separate.
  6. normalize: DVE reciprocal of the denominators, K=2 PE matmul
     broadcast across partitions, one DVE multiply (flash-style deferred
     normalization)
  7. out_partial[tok, 1024] = ctx (stationary) @ WoT slice; evictions on
     ScalarE (DVE is busy with rope), stores batched 4 tiles per DMA

Scheduling: bulk loads/stores ride the sync DMA queue; compute-dependent
moves (rope rotate, table duplication) use the scalar-engine hardware DGE
queue.  Batch-1 input DMA is queued before batch-0 attention and batch-1
projections are interleaved *between* attention phases so the 10-buffer
stage pool keeps recycling and the DMA never goes idle.  Warm-up matmuls
at t~2us open the HAM clock gate before the first projection.
"""

import os
import numpy as np
import ml_dtypes

import concourse.bass as bass
import concourse.mybir as mybir
import concourse.tile as tile
from concourse import bacc
from concourse.bass_utils import run_bass_kernel_spmd

BF = mybir.dt.bfloat16
F32 = mybir.dt.float32
AF = mybir.ActivationFunctionType

NCORES = 8
B = 2
L = 2048
D = 1024          # d_model
H = 16            # heads
DK = 64           # head dim
HPC = H // NCORES  # heads per core = 2
PD = HPC * DK      # projection dims per core = 128
TOK = B * L        # 4096 tokens
P = 128

ROPE_BASE = 10000.0


def build_nc(debug_dumps=False):
    """Build the single-core Bass program (SPMD: same program, per-core data)."""
    from contextlib import ExitStack

    nc = bacc.Bacc("TRN2", target_bir_lowering=False, debug=False)

    # ---- DRAM I/O ----
    qT = nc.dram_tensor("qT", [D, TOK], BF, kind="ExternalInput").ap()
    kT = nc.dram_tensor("kT", [D, TOK], BF, kind="ExternalInput").ap()
    vT = nc.dram_tensor("vT", [D, TOK], BF, kind="ExternalInput").ap()
    wqT = nc.dram_tensor("wqT", [D, PD], BF, kind="ExternalInput").ap()
    wkT = nc.dram_tensor("wkT", [D, PD], BF, kind="ExternalInput").ap()
    wvT = nc.dram_tensor("wvT", [D, PD], BF, kind="ExternalInput").ap()
    woT = nc.dram_tensor("woT", [PD, D], BF, kind="ExternalInput").ap()
    bq_d = nc.dram_tensor("bq", [PD, 1], F32, kind="ExternalInput").ap()
    bk_d = nc.dram_tensor("bk", [PD, 1], F32, kind="ExternalInput").ap()
    bqr_d = nc.dram_tensor("bqr", [PD, 1], F32, kind="ExternalInput").ap()
    bkr_d = nc.dram_tensor("bkr", [PD, 1], F32, kind="ExternalInput").ap()
    # RoPE tables ship as the unique 64 head-dim rows; duplicated on-chip
    cos_q = nc.dram_tensor("cos_q", [DK, L], BF, kind="ExternalInput").ap()
    sin_q = nc.dram_tensor("sin_q", [DK, L], BF, kind="ExternalInput").ap()
    cos_k = nc.dram_tensor("cos_k", [DK, L], BF, kind="ExternalInput").ap()
    sin_k = nc.dram_tensor("sin_k", [DK, L], BF, kind="ExternalInput").ap()
    # ones2 [128, 4]: cols 0/1 = even-chunk head-0/head-1 row selectors,
    # cols 2/3 = odd-chunk selectors (skewed packing swaps the halves)
    ones2_d = nc.dram_tensor("ones2", [P, 4], BF, kind="ExternalInput").ap()
    sel_d = nc.dram_tensor("sel", [2, P], BF, kind="ExternalInput").ap()
    outp = nc.dram_tensor("outp", [TOK, D], BF, kind="ExternalOutput").ap()

    with tile.TileContext(nc) as tc, ExitStack() as ctx:
        const = ctx.enter_context(tc.tile_pool(name="const", bufs=1))
        persist = ctx.enter_context(tc.tile_pool(name="persist", bufs=1))
        stage = ctx.enter_context(tc.tile_pool(name="stage", bufs=10))
        raws = ctx.enter_context(tc.tile_pool(name="raws", bufs=2))
        rots = ctx.enter_context(tc.tile_pool(name="rots", bufs=2))
        expp = ctx.enter_context(tc.tile_pool(name="expp", bufs=3))
        outs = ctx.enter_context(tc.tile_pool(name="outs", bufs=2))
        smalls = ctx.enter_context(tc.tile_pool(name="smalls", bufs=4))
        mmp = ctx.enter_context(tc.tile_pool(name="mmp", bufs=2, space="PSUM"))
        ctxp = ctx.enter_context(tc.tile_pool(name="ctxp", bufs=1, space="PSUM"))
        rsp = ctx.enter_context(tc.tile_pool(name="rsp", bufs=1, space="PSUM"))

        # ---- prewarm the ScalarE exp table set during the initial DMA wait
        warm_in = smalls.tile([1, 1], F32, name="warm_in")
        warm_out = smalls.tile([1, 1], F32, name="warm_out")
        nc.vector.memset(warm_in[:], 0.0)
        nc.scalar.activation(warm_out[:], warm_in[:], AF.Exp)

        def load_w(name, w_d):
            w_sb = const.tile([P, 8 * P], BF, name=name)
            nc.sync.dma_start(
                w_sb.rearrange("p (a m) -> p a m", a=8),
                w_d.rearrange("(a p) m -> p a m", p=P),
            )
            return w_sb

        def load_c(name, t_d):
            """Load the 64 unique table rows, duplicate to partitions 64:128."""
            t_sb = const.tile([P, L], BF, name=name)
            nc.sync.dma_start(t_sb[0:DK, :], t_d[:])
            nc.scalar.dma_start(t_sb[DK:P, :], t_sb[0:DK, :])
            return t_sb

        def load_b(name, b_d):
            b_sb = const.tile([P, 1], F32, name=name)
            nc.sync.dma_start(b_sb[:], b_d[:])
            return b_sb

        ones2 = const.tile([P, 4], BF)
        nc.sync.dma_start(ones2[:], ones2_d[:])
        sel_sb = const.tile([2, P], BF)
        nc.sync.dma_start(sel_sb[:], sel_d[:])
        wq_sb = load_w("wq_sb", wqT)
        bq_sb = load_b("bq_sb", bq_d)

        # persistent activations
        qq_sb = persist.tile([P, TOK], BF)   # roped q-heads  [128 dims, 4096 tok]
        kk_sb = persist.tile([P, TOK], BF)   # roped k-heads
        ctx_sb = persist.tile([P, TOK], BF)  # normalized attention ctx
        # Skew-packed block-diagonal attention operands; chunk c covers 64
        # tokens ktA = 64c..64c+63 per head:
        #   even chunk 2t: [[kh_h0[tokA], 0], [0, kh_h1[tokB]]]
        #   odd chunk 2t+1: [[0, kh_h0[tokB]], [kh_h1[tokA], 0]]  (columns)
        # where tokA/tokB are the halves of token tile t.  The skew keeps
        # every producer->operand copy partition-aligned.
        kh2 = [persist.tile([P, 32 * P], BF, name=f"kh2_{b}") for b in range(B)]
        vh2 = [persist.tile([P, 32 * P], BF, name=f"vh2_{b}") for b in range(B)]
        for t in kh2 + vh2:
            nc.gpsimd.memset(t[:], 0.0)

        # ---- PE warm-up: open the HAM clock gate while input DMA streams.
        # Dummies use the (phase-A-idle) rsp PSUM bank so they never
        # contend with the projection tiles in mmp.
        def dummies(n):
            dmt = rsp.tile([P, 512], F32, name="dum", tag="rs")
            for _ in range(n):
                nc.tensor.matmul(dmt[:], lhsT=wq_sb[:, 0:P],
                                 rhs=wq_sb[:, 0:512],
                                 start=True, stop=True, skip_group_check=True)

        dummies(24)

        # ---------- phase helpers ----------
        def load_x(x_d, g):
            """Queue the 8 input stage tiles for token half g (sync queue)."""
            xts = []
            for kt in range(8):
                xt = stage.tile([P, L], BF, name="xstage", tag="stage")
                nc.sync.dma_start(
                    xt[:], x_d[kt * P:(kt + 1) * P, g * L:(g + 1) * L])
                xts.append(xt)
            return xts

        # rotate-half as a DVE stream shuffle: the host permutes each
        # head's dims to [0:16, 32:48, 16:32, 48:64] so the rope partner
        # always sits 16 partitions away within the same 32-quadrant
        SHUF = [(i + 16) % 32 for i in range(32)]

        def proj_compute(xts, w_sb, bias_sb, bias_rot_sb, g, dst_sb,
                         cos_sb, sin_sb):
            """Project token half g (2048 tokens) and apply RoPE (q/k path)."""
            ps = [mmp.tile([P, 1024], F32, name=f"pj{g}_{half}", tag="mm")
                  for half in range(2)]
            for kt in range(8):
                xt = xts[kt]
                for half in range(2):
                    for nb in range(2):
                        c0 = half * 1024 + nb * 512
                        nc.tensor.matmul(
                            ps[half][:, nb * 512:(nb + 1) * 512],
                            lhsT=w_sb[:, kt * P:(kt + 1) * P],
                            rhs=xt[:, c0:c0 + 512],
                            start=(kt == 0), stop=(kt == 7),
                        )
            dst = dst_sb[:, g * L:(g + 1) * L]
            for half in range(2):
                so = slice(half * 1024, (half + 1) * 1024)
                # shuffle must keep the dtype (s4d4_tr_same_src_dst_type)
                rr = rots.tile([P, 1024], F32, name="rr", tag="rot")
                nc.vector.stream_shuffle(rr[:], ps[half][:], SHUF)
                rc = raws.tile([P, 1024], BF, name="rc", tag="raw")
                # (ps + bias) * cos and (shuffled ps + shuffled bias) * sin
                # (sign folded into the sin table), summed into the dest
                nc.vector.scalar_tensor_tensor(
                    rc[:], ps[half][:], bias_sb[:], cos_sb[:, so],
                    op0=mybir.AluOpType.add, op1=mybir.AluOpType.mult)
                nc.vector.scalar_tensor_tensor(
                    rr[:], rr[:], bias_rot_sb[:], sin_sb[:, so],
                    op0=mybir.AluOpType.add, op1=mybir.AluOpType.mult)
                nc.vector.tensor_add(dst[:, so], rc[:], rr[:])

        def proj_v(xts, g):
            """Token-major v projection straight into vh2[g].

            Per 128-token tile t the PSUM holds [tok, vdim]; the skewed
            diagonal blocks are then partition-aligned sub-copies:
              even chunk 2t:  [0:64, 0:64] = h0 tokA, [64:128, 64:128] = h1 tokB
              odd chunk 2t+1: [0:64, 64:128] = h1 tokA, [64:128, 0:64] = h0 tokB
            (bv is folded into the host-side output bias via Wo @ bv.)
            """
            vh2_b = vh2[g]
            for half in range(2):
                vps = mmp.tile([P, 1024], F32, name="vps", tag="mm")
                for t8 in range(8):
                    t = half * 8 + t8
                    for kt in range(8):
                        nc.tensor.matmul(
                            vps[:, t8 * P:(t8 + 1) * P],
                            lhsT=xts[kt][:, t * P:(t + 1) * P],
                            rhs=wv_sb[:, kt * P:(kt + 1) * P],
                            start=(kt == 0), stop=(kt == 7),
                        )
                # strided casts: windows of 256 elements = one chunk pair
                w0 = half * 8 * 256
                dst = vh2_b[0:DK, w0:w0 + 8 * 256].rearrange(
                    "p (t a u) -> p t a u", t=8, u=DK)
                src = vps[0:DK, :].rearrange("p (t a u) -> p t a u", t=8, u=DK)
                # lower partitions: even-col0 (off 0) and odd-col64 (off 192)
                nc.vector.tensor_copy(dst[:, :, 0::3, :], src[:])
                dstu = vh2_b[DK:P, w0:w0 + 8 * 256].rearrange(
                    "p (t a u) -> p t a u", t=8, u=DK)
                srcu = vps[DK:P, :].rearrange("p (t a u) -> p t a u", t=8, u=DK)
                # upper partitions: even-col64 (off 64) <- h1 cols,
                #                   odd-col0 (off 128) <- h0 cols
                nc.vector.tensor_copy(dstu[:, :, 1:2, :], srcu[:, :, 1:2, :])
                nc.vector.tensor_copy(dstu[:, :, 2:3, :], srcu[:, :, 0:1, :])

        def build_kh2(b):
            """Fill kh2[b] from roped kk (4 partition-aligned strided copies)."""
            kh2_r = kh2[b].rearrange("p (t e u) -> p t e u", t=16, u=P)
            kk_r = kk_sb[:, b * L:(b + 1) * L].rearrange(
                "p (t s u) -> p t s u", t=16, u=DK)
            nc.vector.tensor_copy(kh2_r[0:DK, :, 0:1, 0:DK], kk_r[0:DK, :, 0:1, :])
            nc.vector.tensor_copy(kh2_r[DK:P, :, 0:1, DK:P], kk_r[DK:P, :, 1:2, :])
            nc.vector.tensor_copy(kh2_r[DK:P, :, 1:2, 0:DK], kk_r[DK:P, :, 0:1, :])
            nc.vector.tensor_copy(kh2_r[0:DK, :, 1:2, DK:P], kk_r[0:DK, :, 1:2, :])

        def attention(b, q2):
            """Both heads at once via skew-packed block-diagonal K=128 matmuls."""
            qs = qq_sb[:, b * L + q2 * 1024: b * L + (q2 + 1) * 1024]
            cp = ctxp.tile([P, 1024], F32, name="cp", tag="ctx")
            rs = rsp.tile([2, 1024], F32, name="rs", tag="rs")
            ex_prev = None
            for c in range(33):
                ex_cur = None
                if c < 32:
                    sc = mmp.tile([P, 1024], F32, name="sc", tag="mm")
                    for nb in range(2):
                        nc.tensor.matmul(
                            sc[:, nb * 512:(nb + 1) * 512],
                            lhsT=kh2[b][:, c * P:(c + 1) * P],
                            rhs=qs[:, nb * 512:(nb + 1) * 512],
                            start=True, stop=True, skip_group_check=True,
                        )
                    ex_cur = expp.tile([P, 1024], BF, name="ex", tag="exp")
                    nc.scalar.activation(ex_cur[:], sc[:], AF.Exp)
                if c >= 1:
                    cpv = c - 1
                    for nb in range(2):
                        sl = slice(nb * 512, (nb + 1) * 512)
                        nc.tensor.matmul(
                            cp[:, sl], lhsT=vh2[b][:, cpv * P:(cpv + 1) * P],
                            rhs=ex_prev[:, sl],
                            start=(cpv == 0), stop=(cpv == 31),
                            skip_group_check=True,
                        )
                    o2 = ones2[:, 0:2] if cpv % 2 == 0 else ones2[:, 2:4]
                    for nb in range(2):
                        sl = slice(nb * 512, (nb + 1) * 512)
                        nc.tensor.matmul(
                            rs[:, sl], lhsT=o2, rhs=ex_prev[:, sl],
                            start=(cpv == 0), stop=(cpv == 31),
                            skip_group_check=True,
                        )
                ex_prev = ex_cur
            # normalize: reciprocal of the two denominator rows, broadcast
            # across partitions with a K=2 matmul, one DVE multiply.
            rsum = smalls.tile([2, 1024], F32, name="rsum", tag="rsum", bufs=2)
            nc.vector.tensor_copy(rsum[:], rs[:])
            rec = smalls.tile([2, 1024], F32, name="rec", tag="rec", bufs=2)
            nc.vector.reciprocal_approx_fast(rec[:], rsum[:])
            rec_bf = smalls.tile([2, 1024], BF, name="recbf", tag="recbf", bufs=2)
            nc.vector.tensor_copy(rec_bf[:], rec[:])
            bcs = rsp.tile([P, 1024], F32, name="bcs", tag="rs")
            for nb in range(2):
                sl = slice(nb * 512, (nb + 1) * 512)
                nc.tensor.matmul(
                    bcs[:, sl], lhsT=sel_sb[:], rhs=rec_bf[:, sl],
                    start=True, stop=True, skip_group_check=True,
                )
            craw = smalls.tile([P, 1024], BF, name="craw", tag="craw", bufs=2)
            # ScalarE evicts cp in parallel with the DVE reciprocal chain
            nc.scalar.activation(craw[:], cp[:], AF.Identity)
            c0 = b * L + q2 * 1024
            nc.vector.tensor_mul(ctx_sb[:, c0:c0 + 1024], craw[:], bcs[:])

        def out_proj(b, q2):
            ob = None
            for i, tb in enumerate(range(q2 * 8, q2 * 8 + 8)):
                t0 = b * L + tb * P
                if i % 4 == 0:
                    ob = outs.tile([P, 4 * D], BF, name="ob", tag="out")
                po = mmp.tile([P, D], F32, name="po", tag="mm")
                for nb in range(2):
                    nc.tensor.matmul(
                        po[:, nb * 512:(nb + 1) * 512],
                        lhsT=ctx_sb[:, t0:t0 + P],
                        rhs=wo_sb[:, nb * 512:(nb + 1) * 512],
                        start=True, stop=True, skip_group_check=True,
                    )
                # evict on ScalarE: out_proj never overlaps attention exps
                # in this schedule, and DVE is the normalize/rope engine
                nc.scalar.activation(
                    ob[:, (i % 4) * D:(i % 4 + 1) * D], po[:], AF.Identity)
                if i % 4 == 3:
                    t00 = b * L + (tb - 3) * P
                    nc.sync.dma_start(
                        outp[t00:t00 + 4 * P, :].rearrange(
                            "(a p) d -> p a d", p=P),
                        ob.rearrange("p (a d) -> p a d", a=4))

        # ---------- program ----------
        # phase A order v -> k -> q: the LDW-bound v projection hides
        # under the k/q loads and the shortest post-load tail (q: rope
        # only) lands right before attention
        wv_sb = load_w("wv_sb", wvT)
        xv0 = load_x(vT, 0)
        proj_v(xv0, 0)
        dummies(10)
        wk_sb = load_w("wk_sb", wkT)
        bk_sb = load_b("bk_sb", bk_d)
        bkr_sb = load_b("bkr_sb", bkr_d)
        ck_sb = load_c("ck_sb", cos_k)
        sk_sb = load_c("sk_sb", sin_k)
        xk0 = load_x(kT, 0)
        proj_compute(xk0, wk_sb, bk_sb, bkr_sb, 0, kk_sb, ck_sb, sk_sb)
        build_kh2(0)
        dummies(10)
        bqr_sb = load_b("bqr_sb", bqr_d)
        cq_sb = load_c("cq_sb", cos_q)
        sq_sb = load_c("sq_sb", sin_q)
        xq0 = load_x(qT, 0)
        proj_compute(xq0, wq_sb, bq_sb, bqr_sb, 0, qq_sb, cq_sb, sq_sb)
        wo_sb = const.tile([P, D], BF)
        nc.sync.dma_start(wo_sb[:], woT[:])
        # queue batch-1 input DMA now: it streams on the sync queue while
        # the PE runs batch-0 attention; batch-1 projections are slotted
        # between attention phases so stage buffers keep recycling
        xv1 = load_x(vT, 1)
        xk1 = load_x(kT, 1)
        xq1 = load_x(qT, 1)
        attention(0, 0)
        proj_v(xv1, 1)
        attention(0, 1)
        proj_compute(xk1, wk_sb, bk_sb, bkr_sb, 1, kk_sb, ck_sb, sk_sb)
        build_kh2(1)
        out_proj(0, 0)
        proj_compute(xq1, wq_sb, bq_sb, bqr_sb, 1, qq_sb, cq_sb, sq_sb)
        out_proj(0, 1)
        attention(1, 0)
        attention(1, 1)
        out_proj(1, 0)
        out_proj(1, 1)

    return nc


# Per-head dim permutation making rotate-half a within-32-quadrant
# 16-swap (the device's stream_shuffle): [0:16, 32:48, 16:32, 48:64].
# q and k are permuted identically so attention scores are unchanged.
_PERM64 = np.concatenate([np.arange(0, 16), np.arange(32, 48),
                          np.arange(16, 32), np.arange(48, 64)])
_PERM128 = np.concatenate([_PERM64, _PERM64 + DK])
# the shuffle the device applies: out[j] = in[(j//32)*32 + (j%32+16)%32]
_SHUF128 = np.array([(j // 32) * 32 + (j % 32 + 16) % 32 for j in range(P)])


def _rope_tables():
    """Host-built RoPE tables, transposed to [d, t] (unique 64 rows).

    sin is sign-folded for the rotate-half convention; q tables carry the
    1/sqrt(dk) attention scale.  Rows are dim-permuted to match the
    on-device shuffle layout; the kernel duplicates rows onto partitions
    64:128 on-chip.
    """
    inv_freq = 1.0 / (ROPE_BASE ** (np.arange(0, DK, 2, dtype=np.float64) / DK))
    t = np.arange(L, dtype=np.float64)
    ang = np.outer(t, inv_freq)               # [L, 32]
    emb = np.concatenate([ang, ang], axis=1)  # [L, 64]
    cos = np.cos(emb).T.astype(np.float32)    # [64, L]
    sin = np.sin(emb).T.astype(np.float32)
    sin_folded = sin.copy()
    sin_folded[:32] *= -1.0
    cos = cos[_PERM64]
    sin_folded = sin_folded[_PERM64]
    scale = 1.0 / np.sqrt(DK)
    bf = ml_dtypes.bfloat16
    return (
        (cos * scale).astype(bf), (sin_folded * scale).astype(bf),
        cos.astype(bf), sin_folded.astype(bf),
    )


_NC_CACHE = {}


def _get_nc():
    if "nc" not in _NC_CACHE:
        nc = build_nc()
        nc.finalize()
        _NC_CACHE["nc"] = nc
    return _NC_CACHE["nc"]


def _in_maps(q, k, v, Wq, bq, Wk, bk, Wv, Wo):
    bf = ml_dtypes.bfloat16
    qT = np.ascontiguousarray(q.reshape(TOK, D).T).astype(bf)
    kT = np.ascontiguousarray(k.reshape(TOK, D).T).astype(bf)
    vT = np.ascontiguousarray(v.reshape(TOK, D).T).astype(bf)
    cos_q, sin_q, cos_k, sin_k = _rope_tables()
    # even-chunk selectors: rows 0:64 = h0, rows 64:128 = h1
    # odd-chunk selectors (skewed): rows 0:64 = h1, rows 64:128 = h0
    ones2 = np.zeros((P, 4), bf)
    ones2[0:DK, 0] = 1
    ones2[DK:P, 1] = 1
    ones2[DK:P, 2] = 1
    ones2[0:DK, 3] = 1
    sel = np.zeros((2, P), bf)
    sel[0, 0:DK] = 1
    sel[1, DK:P] = 1
    in_maps = []
    for c in range(NCORES):
        hs = slice(c * PD, (c + 1) * PD)
        # q/k weight rows and biases carry the rope-shuffle dim permutation
        wq_c = np.asarray(Wq[hs, :])[_PERM128]
        wk_c = np.asarray(Wk[hs, :])[_PERM128]
        bq_c = np.asarray(bq[hs], np.float32)[_PERM128]
        bk_c = np.asarray(bk[hs], np.float32)[_PERM128]
        in_maps.append({
            "qT": qT, "kT": kT, "vT": vT,
            "wqT": np.ascontiguousarray(wq_c.T).astype(bf),
            "wkT": np.ascontiguousarray(wk_c.T).astype(bf),
            "wvT": np.ascontiguousarray(Wv[hs, :].T).astype(bf),
            "woT": np.ascontiguousarray(Wo[:, hs].T).astype(bf),
            "bq": bq_c.reshape(PD, 1),
            "bk": bk_c.reshape(PD, 1),
            "bqr": bq_c[_SHUF128].reshape(PD, 1).copy(),
            "bkr": bk_c[_SHUF128].reshape(PD, 1).copy(),
            "cos_q": cos_q, "sin_q": sin_q, "cos_k": cos_k, "sin_k": sin_k,
            "ones2": ones2, "sel": sel,
        })
    return in_maps


def kernel(q, k, v, Wq, bq, Wk, bk, Wv, bv, Wo, bo):
    assert q.shape == (B, L, D) and k.shape == (B, L, D) and v.shape == (B, L, D)
    in_maps = _in_maps(q, k, v, Wq, bq, Wk, bk, Wv, Wo)
    nc = _get_nc()
    res = run_bass_kernel_spmd(nc, in_maps, list(range(NCORES)))
    out = np.zeros((TOK, D), np.float64)
    for r in res.results:
        out += r["outp"].astype(np.float64)
    # bv never touches the device: since the attention weights sum to 1,
    # its contribution is the constant vector Wo @ bv
    out += np.asarray(bo, np.float64)[None, :]
    out += (np.asarray(Wo, np.float64) @ np.asarray(bv, np.float64))[None, :]
    return out.astype(np.float32).reshape(B, L, D)


# revision 29
# speedup vs baseline: 1.1393x; 1.0099x over previous
"""Trainium2 Bass kernel for MultiHeadAttention with RoPE.

Problem: B=2, L=2048, d_model=1024, 16 heads, d_k=64, fp32 in/out.

Sharding (8 cores): tensor-parallel over heads — core c owns heads
{2c, 2c+1}, i.e. a 128-wide slice of the projection output dims.  Every
core reads the full q/k/v activations (transposed + bf16 on host), its
own 128-row slice of Wq/Wk/Wv (pre-transposed) and the matching 128
columns of Wo.  Each core computes its heads' attention output and the
partial d_model-sized output projection; the host sums the 8 partials
and adds bo (+ Wo @ bv, folded out of the device program).

Per-core pipeline (all matmuls bf16, fp32 PSUM accumulation):
  1. q/k projections:  qh.T = WqT.T @ q.T  laid out [128 head-dims, 2048 tok]
     per batch, RoPE applied via partition-shifted DMA copy + 3 DVE ops
     (1/sqrt(dk) scale and rotate-half sign folded into host tables)
  2. v projection runs token-major (stationary = x tile, moving = WvT
     chunk) so its PSUM tiles are already [token, dim] and evict straight
     into the block-diagonal vh2 operand with plain DVE casts — no PE
     transposes, no staging buffer, no serial placement phase
  3. scores.T chunks [kt-pack 128, qt 512] = kh2 (block-diag stationary,
     K=128 so the HAM clock-gate opens to 2.4 GHz) @ qh
  4. exp on ScalarE (no max-subtract: scores ~ N(0,1)), bf16 out
  5. ctx accumulation via block-diag vh2 stationary; parallel [128, 2]
     ones matmuls accumulate both heads' softmax denominators in [2, qt]
     PSUM.  The diagonal packing is *skewed* (odd chunks anti-diagonal)
     so the token-major v eviction never crosses partitions; the ones
     stationary alternates parity to keep head sums separate.
  6. normalize: DVE reciprocal of the denominators, K=2 PE matmul
     broadcast across partitions, one DVE multiply (flash-style deferred
     normalization)
  7. out_partial[tok, 1024] = ctx (stationary) @ WoT slice; evictions on
     ScalarE (DVE is busy with rope), stores batched 4 tiles per DMA

Scheduling: bulk loads/stores ride the sync DMA queue; compute-dependent
moves (rope rotate, table duplication) use the scalar-engine hardware DGE
queue.  Batch-1 input DMA is queued before batch-0 attention and batch-1
projections are interleaved *between* attention phases so the 10-buffer
stage pool keeps recycling and the DMA never goes idle.  Warm-up matmuls
at t~2us open the HAM clock gate before the first projection.
"""

import os
import numpy as np
import ml_dtypes

import concourse.bass as bass
import concourse.mybir as mybir
import concourse.tile as tile
from concourse import bacc
from concourse.bass_utils import run_bass_kernel_spmd

BF = mybir.dt.bfloat16
F32 = mybir.dt.float32
AF = mybir.ActivationFunctionType

NCORES = 8
B = 2
L = 2048
D = 1024          # d_model
H = 16            # heads
DK = 64           # head dim
HPC = H // NCORES  # heads per core = 2
PD = HPC * DK      # projection dims per core = 128
TOK = B * L        # 4096 tokens
P = 128

ROPE_BASE = 10000.0


def build_nc(debug_dumps=False):
    """Build the single-core Bass program (SPMD: same program, per-core data)."""
    from contextlib import ExitStack

    nc = bacc.Bacc("TRN2", target_bir_lowering=False, debug=False)

    # ---- DRAM I/O ----
    qT = nc.dram_tensor("qT", [D, TOK], BF, kind="ExternalInput").ap()
    kT = nc.dram_tensor("kT", [D, TOK], BF, kind="ExternalInput").ap()
    vT = nc.dram_tensor("vT", [D, TOK], BF, kind="ExternalInput").ap()
    wqT = nc.dram_tensor("wqT", [D, PD], BF, kind="ExternalInput").ap()
    wkT = nc.dram_tensor("wkT", [D, PD], BF, kind="ExternalInput").ap()
    wvT = nc.dram_tensor("wvT", [D, PD], BF, kind="ExternalInput").ap()
    woT = nc.dram_tensor("woT", [PD, D], BF, kind="ExternalInput").ap()
    bq_d = nc.dram_tensor("bq", [PD, 1], F32, kind="ExternalInput").ap()
    bk_d = nc.dram_tensor("bk", [PD, 1], F32, kind="ExternalInput").ap()
    bqr_d = nc.dram_tensor("bqr", [PD, 1], F32, kind="ExternalInput").ap()
    bkr_d = nc.dram_tensor("bkr", [PD, 1], F32, kind="ExternalInput").ap()
    # RoPE tables ship as the unique 64 head-dim rows; duplicated on-chip
    cos_q = nc.dram_tensor("cos_q", [DK, L], BF, kind="ExternalInput").ap()
    sin_q = nc.dram_tensor("sin_q", [DK, L], BF, kind="ExternalInput").ap()
    cos_k = nc.dram_tensor("cos_k", [DK, L], BF, kind="ExternalInput").ap()
    sin_k = nc.dram_tensor("sin_k", [DK, L], BF, kind="ExternalInput").ap()
    # ones2 [128, 4]: cols 0/1 = even-chunk head-0/head-1 row selectors,
    # cols 2/3 = odd-chunk selectors (skewed packing swaps the halves)
    ones2_d = nc.dram_tensor("ones2", [P, 4], BF, kind="ExternalInput").ap()
    sel_d = nc.dram_tensor("sel", [2, P], BF, kind="ExternalInput").ap()
    outp = nc.dram_tensor("outp", [TOK, D], BF, kind="ExternalOutput").ap()

    with tile.TileContext(nc) as tc, ExitStack() as ctx:
        const = ctx.enter_context(tc.tile_pool(name="const", bufs=1))
        persist = ctx.enter_context(tc.tile_pool(name="persist", bufs=1))
        stage = ctx.enter_context(tc.tile_pool(name="stage", bufs=10))
        raws = ctx.enter_context(tc.tile_pool(name="raws", bufs=2))
        rots = ctx.enter_context(tc.tile_pool(name="rots", bufs=2))
        expp = ctx.enter_context(tc.tile_pool(name="expp", bufs=3))
        outs = ctx.enter_context(tc.tile_pool(name="outs", bufs=2))
        smalls = ctx.enter_context(tc.tile_pool(name="smalls", bufs=4))
        mmp = ctx.enter_context(tc.tile_pool(name="mmp", bufs=2, space="PSUM"))
        ctxp = ctx.enter_context(tc.tile_pool(name="ctxp", bufs=1, space="PSUM"))
        rsp = ctx.enter_context(tc.tile_pool(name="rsp", bufs=1, space="PSUM"))

        # ---- prewarm the ScalarE exp table set during the initial DMA wait
        warm_in = smalls.tile([1, 1], F32, name="warm_in")
        warm_out = smalls.tile([1, 1], F32, name="warm_out")
        nc.vector.memset(warm_in[:], 0.0)
        nc.scalar.activation(warm_out[:], warm_in[:], AF.Exp)

        def load_w(name, w_d):
            w_sb = const.tile([P, 8 * P], BF, name=name)
            nc.sync.dma_start(
                w_sb.rearrange("p (a m) -> p a m", a=8),
                w_d.rearrange("(a p) m -> p a m", p=P),
            )
            return w_sb

        def load_c(name, t_d):
            """Load the 64 unique table rows, duplicate to partitions 64:128."""
            t_sb = const.tile([P, L], BF, name=name)
            nc.sync.dma_start(t_sb[0:DK, :], t_d[:])
            nc.scalar.dma_start(t_sb[DK:P, :], t_sb[0:DK, :])
            return t_sb

        def load_b(name, b_d):
            b_sb = const.tile([P, 1], F32, name=name)
            nc.sync.dma_start(b_sb[:], b_d[:])
            return b_sb

        ones2 = const.tile([P, 4], BF)
        nc.sync.dma_start(ones2[:], ones2_d[:])
        sel_sb = const.tile([2, P], BF)
        nc.sync.dma_start(sel_sb[:], sel_d[:])
        wq_sb = load_w("wq_sb", wqT)
        bq_sb = load_b("bq_sb", bq_d)

        # persistent activations
        qq_sb = persist.tile([P, TOK], BF)   # roped q-heads  [128 dims, 4096 tok]
        kk_sb = persist.tile([P, TOK], BF)   # roped k-heads
        ctx_sb = persist.tile([P, TOK], BF)  # normalized attention ctx
        # Skew-packed block-diagonal attention operands; chunk c covers 64
        # tokens ktA = 64c..64c+63 per head:
        #   even chunk 2t: [[kh_h0[tokA], 0], [0, kh_h1[tokB]]]
        #   odd chunk 2t+1: [[0, kh_h0[tokB]], [kh_h1[tokA], 0]]  (columns)
        # where tokA/tokB are the halves of token tile t.  The skew keeps
        # every producer->operand copy partition-aligned.
        kh2 = [persist.tile([P, 32 * P], BF, name=f"kh2_{b}") for b in range(B)]
        vh2 = [persist.tile([P, 32 * P], BF, name=f"vh2_{b}") for b in range(B)]
        for t in kh2 + vh2:
            nc.gpsimd.memset(t[:], 0.0)

        # ---- PE warm-up: open the HAM clock gate while input DMA streams.
        # Dummies use the (phase-A-idle) rsp PSUM bank so they never
        # contend with the projection tiles in mmp.
        def dummies(n):
            dmt = rsp.tile([P, 512], F32, name="dum", tag="rs")
            for _ in range(n):
                nc.tensor.matmul(dmt[:], lhsT=wq_sb[:, 0:P],
                                 rhs=wq_sb[:, 0:512],
                                 start=True, stop=True, skip_group_check=True)

        dummies(24)

        # ---------- phase helpers ----------
        def load_x(x_d, g):
            """Queue the 8 input stage tiles for token half g (sync queue)."""
            xts = []
            for kt in range(8):
                xt = stage.tile([P, L], BF, name="xstage", tag="stage")
                nc.sync.dma_start(
                    xt[:], x_d[kt * P:(kt + 1) * P, g * L:(g + 1) * L])
                xts.append(xt)
            return xts

        # rotate-half as a DVE stream shuffle: the host permutes each
        # head's dims to [0:16, 32:48, 16:32, 48:64] so the rope partner
        # always sits 16 partitions away within the same 32-quadrant
        SHUF = [(i + 16) % 32 for i in range(32)]

        def proj_compute(xts, w_sb, bias_sb, bias_rot_sb, g, dst_sb,
                         cos_sb, sin_sb):
            """Project token half g (2048 tokens) and apply RoPE (q/k path)."""
            ps = [mmp.tile([P, 1024], F32, name=f"pj{g}_{half}", tag="mm")
                  for half in range(2)]
            for kt in range(8):
                xt = xts[kt]
                for half in range(2):
                    for nb in range(2):
                        c0 = half * 1024 + nb * 512
                        nc.tensor.matmul(
                            ps[half][:, nb * 512:(nb + 1) * 512],
                            lhsT=w_sb[:, kt * P:(kt + 1) * P],
                            rhs=xt[:, c0:c0 + 512],
                            start=(kt == 0), stop=(kt == 7),
                        )
            dst = dst_sb[:, g * L:(g + 1) * L]
            for half in range(2):
                so = slice(half * 1024, (half + 1) * 1024)
                # shuffle must keep the dtype (s4d4_tr_same_src_dst_type)
                rr = rots.tile([P, 1024], F32, name="rr", tag="rot")
                nc.vector.stream_shuffle(rr[:], ps[half][:], SHUF)
                rc = raws.tile([P, 1024], BF, name="rc", tag="raw")
                # (ps + bias) * cos and (shuffled ps + shuffled bias) * sin
                # (sign folded into the sin table), summed into the dest
                nc.vector.scalar_tensor_tensor(
                    rc[:], ps[half][:], bias_sb[:], cos_sb[:, so],
                    op0=mybir.AluOpType.add, op1=mybir.AluOpType.mult)
                nc.vector.scalar_tensor_tensor(
                    rr[:], rr[:], bias_rot_sb[:], sin_sb[:, so],
                    op0=mybir.AluOpType.add, op1=mybir.AluOpType.mult)
                nc.vector.tensor_add(dst[:, so], rc[:], rr[:])

        def proj_v(xts, g):
            """Token-major v projection straight into vh2[g].

            Per 128-token tile t the PSUM holds [tok, vdim]; the skewed
            diagonal blocks are then partition-aligned sub-copies:
              even chunk 2t:  [0:64, 0:64] = h0 tokA, [64:128, 64:128] = h1 tokB
              odd chunk 2t+1: [0:64, 64:128] = h1 tokA, [64:128, 0:64] = h0 tokB
            (bv is folded into the host-side output bias via Wo @ bv.)
            """
            vh2_b = vh2[g]
            for half in range(2):
                vps = mmp.tile([P, 1024], F32, name="vps", tag="mm")
                for t8 in range(8):
                    t = half * 8 + t8
                    for kt in range(8):
                        nc.tensor.matmul(
                            vps[:, t8 * P:(t8 + 1) * P],
                            lhsT=xts[kt][:, t * P:(t + 1) * P],
                            rhs=wv_sb[:, kt * P:(kt + 1) * P],
                            start=(kt == 0), stop=(kt == 7),
                        )
                # strided casts: windows of 256 elements = one chunk pair
                w0 = half * 8 * 256
                dst = vh2_b[0:DK, w0:w0 + 8 * 256].rearrange(
                    "p (t a u) -> p t a u", t=8, u=DK)
                src = vps[0:DK, :].rearrange("p (t a u) -> p t a u", t=8, u=DK)
                # lower partitions: even-col0 (off 0) and odd-col64 (off 192)
                nc.vector.tensor_copy(dst[:, :, 0::3, :], src[:])
                dstu = vh2_b[DK:P, w0:w0 + 8 * 256].rearrange(
                    "p (t a u) -> p t a u", t=8, u=DK)
                srcu = vps[DK:P, :].rearrange("p (t a u) -> p t a u", t=8, u=DK)
                # upper partitions: even-col64 (off 64) <- h1 cols,
                #                   odd-col0 (off 128) <- h0 cols
                nc.vector.tensor_copy(dstu[:, :, 1:2, :], srcu[:, :, 1:2, :])
                nc.vector.tensor_copy(dstu[:, :, 2:3, :], srcu[:, :, 0:1, :])

        def build_kh2(b):
            """Fill kh2[b] from roped kk (4 partition-aligned strided copies)."""
            kh2_r = kh2[b].rearrange("p (t e u) -> p t e u", t=16, u=P)
            kk_r = kk_sb[:, b * L:(b + 1) * L].rearrange(
                "p (t s u) -> p t s u", t=16, u=DK)
            nc.vector.tensor_copy(kh2_r[0:DK, :, 0:1, 0:DK], kk_r[0:DK, :, 0:1, :])
            nc.vector.tensor_copy(kh2_r[DK:P, :, 0:1, DK:P], kk_r[DK:P, :, 1:2, :])
            nc.vector.tensor_copy(kh2_r[DK:P, :, 1:2, 0:DK], kk_r[DK:P, :, 0:1, :])
            nc.vector.tensor_copy(kh2_r[0:DK, :, 1:2, DK:P], kk_r[0:DK, :, 1:2, :])

        def attention(b, q2):
            """Both heads at once via skew-packed block-diagonal K=128 matmuls."""
            qs = qq_sb[:, b * L + q2 * 1024: b * L + (q2 + 1) * 1024]
            cp = ctxp.tile([P, 1024], F32, name="cp", tag="ctx")
            rs = rsp.tile([2, 1024], F32, name="rs", tag="rs")
            ex_prev = None
            for c in range(33):
                ex_cur = None
                if c < 32:
                    sc = mmp.tile([P, 1024], F32, name="sc", tag="mm")
                    for nb in range(2):
                        nc.tensor.matmul(
                            sc[:, nb * 512:(nb + 1) * 512],
                            lhsT=kh2[b][:, c * P:(c + 1) * P],
                            rhs=qs[:, nb * 512:(nb + 1) * 512],
                            start=True, stop=True, skip_group_check=True,
                        )
                    ex_cur = expp.tile([P, 1024], BF, name="ex", tag="exp")
                    nc.scalar.activation(ex_cur[:], sc[:], AF.Exp)
                if c >= 1:
                    cpv = c - 1
                    for nb in range(2):
                        sl = slice(nb * 512, (nb + 1) * 512)
                        nc.tensor.matmul(
                            cp[:, sl], lhsT=vh2[b][:, cpv * P:(cpv + 1) * P],
                            rhs=ex_prev[:, sl],
                            start=(cpv == 0), stop=(cpv == 31),
                            skip_group_check=True,
                        )
                    o2 = ones2[:, 0:2] if cpv % 2 == 0 else ones2[:, 2:4]
                    for nb in range(2):
                        sl = slice(nb * 512, (nb + 1) * 512)
                        nc.tensor.matmul(
                            rs[:, sl], lhsT=o2, rhs=ex_prev[:, sl],
                            start=(cpv == 0), stop=(cpv == 31),
                            skip_group_check=True,
                        )
                ex_prev = ex_cur
            # normalize: reciprocal of the two denominator rows, broadcast
            # across partitions with a K=2 matmul, one DVE multiply.
            rsum = smalls.tile([2, 1024], F32, name="rsum", tag="rsum", bufs=2)
            nc.vector.tensor_copy(rsum[:], rs[:])
            rec = smalls.tile([2, 1024], F32, name="rec", tag="rec", bufs=2)
            nc.vector.reciprocal_approx_fast(rec[:], rsum[:])
            rec_bf = smalls.tile([2, 1024], BF, name="recbf", tag="recbf", bufs=2)
            nc.vector.tensor_copy(rec_bf[:], rec[:])
            bcs = rsp.tile([P, 1024], F32, name="bcs", tag="rs")
            for nb in range(2):
                sl = slice(nb * 512, (nb + 1) * 512)
                nc.tensor.matmul(
                    bcs[:, sl], lhsT=sel_sb[:], rhs=rec_bf[:, sl],
                    start=True, stop=True, skip_group_check=True,
                )
            craw = smalls.tile([P, 1024], BF, name="craw", tag="craw", bufs=2)
            # ScalarE evicts cp in parallel with the DVE reciprocal chain
            nc.scalar.activation(craw[:], cp[:], AF.Identity)
            c0 = b * L + q2 * 1024
            nc.vector.tensor_mul(ctx_sb[:, c0:c0 + 1024], craw[:], bcs[:])

        def out_proj(b, q2, pool=None):
            # mid-section calls pass pool=ctxp: their PSUM tiles must not
            # rotate through mmp, whose slots are pinned by in-flight
            # projection tiles until the rope chain reads them
            pool = pool or mmp
            tag = "ctx" if pool is ctxp else "mm"
            ob = None
            for i, tb in enumerate(range(q2 * 8, q2 * 8 + 8)):
                t0 = b * L + tb * P
                if i % 4 == 0:
                    ob = outs.tile([P, 4 * D], BF, name="ob", tag="out")
                po = pool.tile([P, D], F32, name="po", tag=tag)
                for nb in range(2):
                    nc.tensor.matmul(
                        po[:, nb * 512:(nb + 1) * 512],
                        lhsT=ctx_sb[:, t0:t0 + P],
                        rhs=wo_sb[:, nb * 512:(nb + 1) * 512],
                        start=True, stop=True, skip_group_check=True,
                    )
                # evict on ScalarE: out_proj never overlaps attention exps
                # in this schedule, and DVE is the normalize/rope engine
                nc.scalar.activation(
                    ob[:, (i % 4) * D:(i % 4 + 1) * D], po[:], AF.Identity)
                if i % 4 == 3:
                    t00 = b * L + (tb - 3) * P
                    nc.sync.dma_start(
                        outp[t00:t00 + 4 * P, :].rearrange(
                            "(a p) d -> p a d", p=P),
                        ob.rearrange("p (a d) -> p a d", a=4))

        # ---------- program ----------
        # phase A order v -> k -> q: the LDW-bound v projection hides
        # under the k/q loads and the shortest post-load tail (q: rope
        # only) lands right before attention
        wv_sb = load_w("wv_sb", wvT)
        xv0 = load_x(vT, 0)
        proj_v(xv0, 0)
        dummies(10)
        wk_sb = load_w("wk_sb", wkT)
        bk_sb = load_b("bk_sb", bk_d)
        bkr_sb = load_b("bkr_sb", bkr_d)
        ck_sb = load_c("ck_sb", cos_k)
        sk_sb = load_c("sk_sb", sin_k)
        xk0 = load_x(kT, 0)
        proj_compute(xk0, wk_sb, bk_sb, bkr_sb, 0, kk_sb, ck_sb, sk_sb)
        build_kh2(0)
        dummies(10)
        bqr_sb = load_b("bqr_sb", bqr_d)
        cq_sb = load_c("cq_sb", cos_q)
        sq_sb = load_c("sq_sb", sin_q)
        xq0 = load_x(qT, 0)
        proj_compute(xq0, wq_sb, bq_sb, bqr_sb, 0, qq_sb, cq_sb, sq_sb)
        # keep the PE busy (and the clock gate open) across the rope-q
        # chain so attention(0,0) starts warm
        dummies(20)
        wo_sb = const.tile([P, D], BF)
        nc.sync.dma_start(wo_sb[:], woT[:])
        # queue batch-1 input DMA now: it streams on the sync queue while
        # the PE runs batch-0 attention; batch-1 projections are slotted
        # between attention phases so stage buffers keep recycling
        xv1 = load_x(vT, 1)
        xk1 = load_x(kT, 1)
        xq1 = load_x(qT, 1)
        attention(0, 0)
        proj_v(xv1, 1)
        attention(0, 1)
        proj_compute(xk1, wk_sb, bk_sb, bkr_sb, 1, kk_sb, ck_sb, sk_sb)
        build_kh2(1)
        out_proj(0, 0, pool=ctxp)
        proj_compute(xq1, wq_sb, bq_sb, bqr_sb, 1, qq_sb, cq_sb, sq_sb)
        out_proj(0, 1, pool=ctxp)
        attention(1, 0)
        attention(1, 1)
        out_proj(1, 0)
        out_proj(1, 1)

    return nc


# Per-head dim permutation making rotate-half a within-32-quadrant
# 16-swap (the device's stream_shuffle): [0:16, 32:48, 16:32, 48:64].
# q and k are permuted identically so attention scores are unchanged.
_PERM64 = np.concatenate([np.arange(0, 16), np.arange(32, 48),
                          np.arange(16, 32), np.arange(48, 64)])
_PERM128 = np.concatenate([_PERM64, _PERM64 + DK])
# the shuffle the device applies: out[j] = in[(j//32)*32 + (j%32+16)%32]
_SHUF128 = np.array([(j // 32) * 32 + (j % 32 + 16) % 32 for j in range(P)])


def _rope_tables():
    """Host-built RoPE tables, transposed to [d, t] (unique 64 rows).

    sin is sign-folded for the rotate-half convention; q tables carry the
    1/sqrt(dk) attention scale.  Rows are dim-permuted to match the
    on-device shuffle layout; the kernel duplicates rows onto partitions
    64:128 on-chip.
    """
    inv_freq = 1.0 / (ROPE_BASE ** (np.arange(0, DK, 2, dtype=np.float64) / DK))
    t = np.arange(L, dtype=np.float64)
    ang = np.outer(t, inv_freq)               # [L, 32]
    emb = np.concatenate([ang, ang], axis=1)  # [L, 64]
    cos = np.cos(emb).T.astype(np.float32)    # [64, L]
    sin = np.sin(emb).T.astype(np.float32)
    sin_folded = sin.copy()
    sin_folded[:32] *= -1.0
    cos = cos[_PERM64]
    sin_folded = sin_folded[_PERM64]
    scale = 1.0 / np.sqrt(DK)
    bf = ml_dtypes.bfloat16
    return (
        (cos * scale).astype(bf), (sin_folded * scale).astype(bf),
        cos.astype(bf), sin_folded.astype(bf),
    )


_NC_CACHE = {}


def _get_nc():
    if "nc" not in _NC_CACHE:
        nc = build_nc()
        nc.finalize()
        _NC_CACHE["nc"] = nc
    return _NC_CACHE["nc"]


def _in_maps(q, k, v, Wq, bq, Wk, bk, Wv, Wo):
    bf = ml_dtypes.bfloat16
    qT = np.ascontiguousarray(q.reshape(TOK, D).T).astype(bf)
    kT = np.ascontiguousarray(k.reshape(TOK, D).T).astype(bf)
    vT = np.ascontiguousarray(v.reshape(TOK, D).T).astype(bf)
    cos_q, sin_q, cos_k, sin_k = _rope_tables()
    # even-chunk selectors: rows 0:64 = h0, rows 64:128 = h1
    # odd-chunk selectors (skewed): rows 0:64 = h1, rows 64:128 = h0
    ones2 = np.zeros((P, 4), bf)
    ones2[0:DK, 0] = 1
    ones2[DK:P, 1] = 1
    ones2[DK:P, 2] = 1
    ones2[0:DK, 3] = 1
    sel = np.zeros((2, P), bf)
    sel[0, 0:DK] = 1
    sel[1, DK:P] = 1
    in_maps = []
    for c in range(NCORES):
        hs = slice(c * PD, (c + 1) * PD)
        # q/k weight rows and biases carry the rope-shuffle dim permutation
        wq_c = np.asarray(Wq[hs, :])[_PERM128]
        wk_c = np.asarray(Wk[hs, :])[_PERM128]
        bq_c = np.asarray(bq[hs], np.float32)[_PERM128]
        bk_c = np.asarray(bk[hs], np.float32)[_PERM128]
        in_maps.append({
            "qT": qT, "kT": kT, "vT": vT,
            "wqT": np.ascontiguousarray(wq_c.T).astype(bf),
            "wkT": np.ascontiguousarray(wk_c.T).astype(bf),
            "wvT": np.ascontiguousarray(Wv[hs, :].T).astype(bf),
            "woT": np.ascontiguousarray(Wo[:, hs].T).astype(bf),
            "bq": bq_c.reshape(PD, 1),
            "bk": bk_c.reshape(PD, 1),
            "bqr": bq_c[_SHUF128].reshape(PD, 1).copy(),
            "bkr": bk_c[_SHUF128].reshape(PD, 1).copy(),
            "cos_q": cos_q, "sin_q": sin_q, "cos_k": cos_k, "sin_k": sin_k,
            "ones2": ones2, "sel": sel,
        })
    return in_maps


def kernel(q, k, v, Wq, bq, Wk, bk, Wv, bv, Wo, bo):
    assert q.shape == (B, L, D) and k.shape == (B, L, D) and v.shape == (B, L, D)
    in_maps = _in_maps(q, k, v, Wq, bq, Wk, bk, Wv, Wo)
    nc = _get_nc()
    res = run_bass_kernel_spmd(nc, in_maps, list(range(NCORES)))
    out = np.zeros((TOK, D), np.float64)
    for r in res.results:
        out += r["outp"].astype(np.float64)
    # bv never touches the device: since the attention weights sum to 1,
    # its contribution is the constant vector Wo @ bv
    out += np.asarray(bo, np.float64)[None, :]
    out += (np.asarray(Wo, np.float64) @ np.asarray(bv, np.float64))[None, :]
    return out.astype(np.float32).reshape(B, L, D)


# revision 37
# speedup vs baseline: 1.1530x; 1.0121x over previous
"""Trainium2 Bass kernel for MultiHeadAttention with RoPE.

Problem: B=2, L=2048, d_model=1024, 16 heads, d_k=64, fp32 in/out.

Sharding (8 cores): tensor-parallel over heads — core c owns heads
{2c, 2c+1}, i.e. a 128-wide slice of the projection output dims.  Every
core reads the full q/k/v activations (transposed + bf16 on host), its
own 128-row slice of Wq/Wk/Wv (pre-transposed) and the matching 128
columns of Wo.  Each core computes its heads' attention output and the
partial d_model-sized output projection; the host sums the 8 partials
and adds bo (+ Wo @ bv, folded out of the device program).

Per-core pipeline (all matmuls bf16, fp32 PSUM accumulation):
  1. q/k projections:  qh.T = WqT.T @ q.T  laid out [128 head-dims, 2048 tok]
     per batch, RoPE applied via partition-shifted DMA copy + 3 DVE ops
     (1/sqrt(dk) scale and rotate-half sign folded into host tables)
  2. v projection runs token-major (stationary = x tile, moving = WvT
     chunk) so its PSUM tiles are already [token, dim] and evict straight
     into the block-diagonal vh2 operand with plain DVE casts — no PE
     transposes, no staging buffer, no serial placement phase
  3. scores.T chunks [kt-pack 128, qt 512] = kh2 (block-diag stationary,
     K=128 so the HAM clock-gate opens to 2.4 GHz) @ qh
  4. exp on ScalarE (no max-subtract: scores ~ N(0,1)), bf16 out
  5. ctx accumulation via block-diag vh2 stationary; parallel [128, 2]
     ones matmuls accumulate both heads' softmax denominators in [2, qt]
     PSUM.  The diagonal packing is *skewed* (odd chunks anti-diagonal)
     so the token-major v eviction never crosses partitions; the ones
     stationary alternates parity to keep head sums separate.
  6. normalize: DVE reciprocal of the denominators, K=2 PE matmul
     broadcast across partitions, one DVE multiply (flash-style deferred
     normalization)
  7. out_partial[tok, 1024] = ctx (stationary) @ WoT slice; evictions on
     ScalarE (DVE is busy with rope), stores batched 4 tiles per DMA

Scheduling: bulk loads/stores ride the sync DMA queue; compute-dependent
moves (rope rotate, table duplication) use the scalar-engine hardware DGE
queue.  Batch-1 input DMA is queued before batch-0 attention and batch-1
projections are interleaved *between* attention phases so the 10-buffer
stage pool keeps recycling and the DMA never goes idle.  Warm-up matmuls
at t~2us open the HAM clock gate before the first projection.
"""

import os
import numpy as np
import ml_dtypes

import concourse.bass as bass
import concourse.mybir as mybir
import concourse.tile as tile
from concourse import bacc
from concourse.bass_utils import run_bass_kernel_spmd

BF = mybir.dt.bfloat16
F32 = mybir.dt.float32
AF = mybir.ActivationFunctionType

NCORES = 8
B = 2
L = 2048
D = 1024          # d_model
H = 16            # heads
DK = 64           # head dim
HPC = H // NCORES  # heads per core = 2
PD = HPC * DK      # projection dims per core = 128
TOK = B * L        # 4096 tokens
P = 128

ROPE_BASE = 10000.0


def build_nc(debug_dumps=False):
    """Build the single-core Bass program (SPMD: same program, per-core data)."""
    from contextlib import ExitStack

    nc = bacc.Bacc("TRN2", target_bir_lowering=False, debug=False)

    # ---- DRAM I/O ----
    qT = nc.dram_tensor("qT", [D, TOK], BF, kind="ExternalInput").ap()
    kT = nc.dram_tensor("kT", [D, TOK], BF, kind="ExternalInput").ap()
    vT = nc.dram_tensor("vT", [D, TOK], BF, kind="ExternalInput").ap()
    wqT = nc.dram_tensor("wqT", [D, PD], BF, kind="ExternalInput").ap()
    wkT = nc.dram_tensor("wkT", [D, PD], BF, kind="ExternalInput").ap()
    wvT = nc.dram_tensor("wvT", [D, PD], BF, kind="ExternalInput").ap()
    woT = nc.dram_tensor("woT", [PD, D], BF, kind="ExternalInput").ap()
    bq_d = nc.dram_tensor("bq", [PD, 1], F32, kind="ExternalInput").ap()
    bk_d = nc.dram_tensor("bk", [PD, 1], F32, kind="ExternalInput").ap()
    bqr_d = nc.dram_tensor("bqr", [PD, 1], F32, kind="ExternalInput").ap()
    bkr_d = nc.dram_tensor("bkr", [PD, 1], F32, kind="ExternalInput").ap()
    # RoPE tables ship as the unique 64 head-dim rows; duplicated on-chip
    cos_q = nc.dram_tensor("cos_q", [DK, L], BF, kind="ExternalInput").ap()
    sin_q = nc.dram_tensor("sin_q", [DK, L], BF, kind="ExternalInput").ap()
    cos_k = nc.dram_tensor("cos_k", [DK, L], BF, kind="ExternalInput").ap()
    sin_k = nc.dram_tensor("sin_k", [DK, L], BF, kind="ExternalInput").ap()
    # ones2 [128, 4]: cols 0/1 = even-chunk head-0/head-1 row selectors,
    # cols 2/3 = odd-chunk selectors (skewed packing swaps the halves)
    ones2_d = nc.dram_tensor("ones2", [P, 4], BF, kind="ExternalInput").ap()
    ones2f_d = nc.dram_tensor("ones2f", [P, 2], F32, kind="ExternalInput").ap()
    sel_d = nc.dram_tensor("sel", [2, P], BF, kind="ExternalInput").ap()
    outp = nc.dram_tensor("outp", [TOK, D], BF, kind="ExternalOutput").ap()

    with tile.TileContext(nc) as tc, ExitStack() as ctx:
        const = ctx.enter_context(tc.tile_pool(name="const", bufs=1))
        persist = ctx.enter_context(tc.tile_pool(name="persist", bufs=1))
        stage = ctx.enter_context(tc.tile_pool(name="stage", bufs=10))
        raws = ctx.enter_context(tc.tile_pool(name="raws", bufs=2))
        rots = ctx.enter_context(tc.tile_pool(name="rots", bufs=2))
        expp = ctx.enter_context(tc.tile_pool(name="expp", bufs=3))
        outs = ctx.enter_context(tc.tile_pool(name="outs", bufs=2))
        smalls = ctx.enter_context(tc.tile_pool(name="smalls", bufs=4))
        mmp = ctx.enter_context(tc.tile_pool(name="mmp", bufs=2, space="PSUM"))
        ctxp = ctx.enter_context(tc.tile_pool(name="ctxp", bufs=1, space="PSUM"))
        rsp = ctx.enter_context(tc.tile_pool(name="rsp", bufs=1, space="PSUM"))

        # ---- prewarm the ScalarE exp table set during the initial DMA wait
        warm_in = smalls.tile([1, 1], F32, name="warm_in")
        warm_out = smalls.tile([1, 1], F32, name="warm_out")
        nc.vector.memset(warm_in[:], 0.0)
        nc.scalar.activation(warm_out[:], warm_in[:], AF.Exp)

        def load_w(name, w_d):
            w_sb = const.tile([P, 8 * P], BF, name=name)
            nc.sync.dma_start(
                w_sb.rearrange("p (a m) -> p a m", a=8),
                w_d.rearrange("(a p) m -> p a m", p=P),
            )
            return w_sb

        def load_c(name, t_d):
            """Load the 64 unique table rows, duplicate to partitions 64:128."""
            t_sb = const.tile([P, L], BF, name=name)
            nc.sync.dma_start(t_sb[0:DK, :], t_d[:])
            nc.scalar.dma_start(t_sb[DK:P, :], t_sb[0:DK, :])
            return t_sb

        def load_b(name, b_d):
            b_sb = const.tile([P, 1], F32, name=name)
            nc.sync.dma_start(b_sb[:], b_d[:])
            return b_sb

        ones2 = const.tile([P, 4], BF)
        nc.sync.dma_start(ones2[:], ones2_d[:])
        # fp32 odd-parity selector for the DVE-accumulated denominator fold
        ones2f = const.tile([P, 2], F32)
        nc.sync.dma_start(ones2f[:], ones2f_d[:])
        sel_sb = const.tile([2, P], BF)
        nc.sync.dma_start(sel_sb[:], sel_d[:])
        wq_sb = load_w("wq_sb", wqT)
        bq_sb = load_b("bq_sb", bq_d)

        # persistent activations
        qq_sb = persist.tile([P, TOK], BF)   # roped q-heads  [128 dims, 4096 tok]
        kk_sb = persist.tile([P, TOK], BF)   # roped k-heads
        ctx_sb = persist.tile([P, TOK], BF)  # normalized attention ctx
        # Skew-packed block-diagonal attention operands; chunk c covers 64
        # tokens ktA = 64c..64c+63 per head:
        #   even chunk 2t: [[kh_h0[tokA], 0], [0, kh_h1[tokB]]]
        #   odd chunk 2t+1: [[0, kh_h0[tokB]], [kh_h1[tokA], 0]]  (columns)
        # where tokA/tokB are the halves of token tile t.  The skew keeps
        # every producer->operand copy partition-aligned.
        kh2 = [persist.tile([P, 32 * P], BF, name=f"kh2_{b}") for b in range(B)]
        vh2 = [persist.tile([P, 32 * P], BF, name=f"vh2_{b}") for b in range(B)]
        for t in kh2 + vh2:
            nc.gpsimd.memset(t[:], 0.0)

        # ---- PE warm-up: open the HAM clock gate while input DMA streams.
        # Dummies use the (phase-A-idle) rsp PSUM bank so they never
        # contend with the projection tiles in mmp.
        def dummies(n):
            dmt = rsp.tile([P, 512], F32, name="dum", tag="rs")
            for _ in range(n):
                nc.tensor.matmul(dmt[:], lhsT=wq_sb[:, 0:P],
                                 rhs=wq_sb[:, 0:512],
                                 start=True, stop=True, skip_group_check=True)

        dummies(24)

        # ---------- phase helpers ----------
        def load_x(x_d, g):
            """Queue the 8 input stage tiles for token half g (sync queue)."""
            xts = []
            for kt in range(8):
                xt = stage.tile([P, L], BF, name="xstage", tag="stage")
                nc.sync.dma_start(
                    xt[:], x_d[kt * P:(kt + 1) * P, g * L:(g + 1) * L])
                xts.append(xt)
            return xts

        # rotate-half as a DVE stream shuffle: the host permutes each
        # head's dims to [0:16, 32:48, 16:32, 48:64] so the rope partner
        # always sits 16 partitions away within the same 32-quadrant
        SHUF = [(i + 16) % 32 for i in range(32)]

        def proj_compute(xts, w_sb, bias_sb, bias_rot_sb, g, dst_sb,
                         cos_sb, sin_sb):
            """Project token half g (2048 tokens) and apply RoPE (q/k path)."""
            ps = [mmp.tile([P, 1024], F32, name=f"pj{g}_{half}", tag="mm")
                  for half in range(2)]
            for kt in range(8):
                xt = xts[kt]
                for half in range(2):
                    for nb in range(2):
                        c0 = half * 1024 + nb * 512
                        nc.tensor.matmul(
                            ps[half][:, nb * 512:(nb + 1) * 512],
                            lhsT=w_sb[:, kt * P:(kt + 1) * P],
                            rhs=xt[:, c0:c0 + 512],
                            start=(kt == 0), stop=(kt == 7),
                        )
            dst = dst_sb[:, g * L:(g + 1) * L]
            for half in range(2):
                so = slice(half * 1024, (half + 1) * 1024)
                # shuffle must keep the dtype (s4d4_tr_same_src_dst_type)
                rr = rots.tile([P, 1024], F32, name="rr", tag="rot")
                nc.vector.stream_shuffle(rr[:], ps[half][:], SHUF)
                rc = raws.tile([P, 1024], BF, name="rc", tag="raw")
                # (ps + bias) * cos and (shuffled ps + shuffled bias) * sin
                # (sign folded into the sin table), summed into the dest
                nc.vector.scalar_tensor_tensor(
                    rc[:], ps[half][:], bias_sb[:], cos_sb[:, so],
                    op0=mybir.AluOpType.add, op1=mybir.AluOpType.mult)
                nc.vector.scalar_tensor_tensor(
                    rr[:], rr[:], bias_rot_sb[:], sin_sb[:, so],
                    op0=mybir.AluOpType.add, op1=mybir.AluOpType.mult)
                nc.vector.tensor_add(dst[:, so], rc[:], rr[:])

        def proj_v(xts, g):
            """Token-major v projection straight into vh2[g].

            Per 128-token tile t the PSUM holds [tok, vdim]; the skewed
            diagonal blocks are then partition-aligned sub-copies:
              even chunk 2t:  [0:64, 0:64] = h0 tokA, [64:128, 64:128] = h1 tokB
              odd chunk 2t+1: [0:64, 64:128] = h1 tokA, [64:128, 0:64] = h0 tokB
            (bv is folded into the host-side output bias via Wo @ bv.)
            """
            vh2_b = vh2[g]
            for half in range(2):
                vps = mmp.tile([P, 1024], F32, name="vps", tag="mm")
                for t8 in range(8):
                    t = half * 8 + t8
                    for kt in range(8):
                        nc.tensor.matmul(
                            vps[:, t8 * P:(t8 + 1) * P],
                            lhsT=xts[kt][:, t * P:(t + 1) * P],
                            rhs=wv_sb[:, kt * P:(kt + 1) * P],
                            start=(kt == 0), stop=(kt == 7),
                        )
                # strided casts: windows of 256 elements = one chunk pair
                w0 = half * 8 * 256
                dst = vh2_b[0:DK, w0:w0 + 8 * 256].rearrange(
                    "p (t a u) -> p t a u", t=8, u=DK)
                src = vps[0:DK, :].rearrange("p (t a u) -> p t a u", t=8, u=DK)
                # lower partitions: even-col0 (off 0) and odd-col64 (off 192)
                nc.vector.tensor_copy(dst[:, :, 0::3, :], src[:])
                dstu = vh2_b[DK:P, w0:w0 + 8 * 256].rearrange(
                    "p (t a u) -> p t a u", t=8, u=DK)
                srcu = vps[DK:P, :].rearrange("p (t a u) -> p t a u", t=8, u=DK)
                # upper partitions: even-col64 (off 64) <- h1 cols,
                #                   odd-col0 (off 128) <- h0 cols
                nc.vector.tensor_copy(dstu[:, :, 1:2, :], srcu[:, :, 1:2, :])
                nc.vector.tensor_copy(dstu[:, :, 2:3, :], srcu[:, :, 0:1, :])

        def build_kh2(b):
            """Fill kh2[b] from roped kk (4 partition-aligned strided copies)."""
            kh2_r = kh2[b].rearrange("p (t e u) -> p t e u", t=16, u=P)
            kk_r = kk_sb[:, b * L:(b + 1) * L].rearrange(
                "p (t s u) -> p t s u", t=16, u=DK)
            nc.vector.tensor_copy(kh2_r[0:DK, :, 0:1, 0:DK], kk_r[0:DK, :, 0:1, :])
            nc.vector.tensor_copy(kh2_r[DK:P, :, 0:1, DK:P], kk_r[DK:P, :, 1:2, :])
            nc.vector.tensor_copy(kh2_r[DK:P, :, 1:2, 0:DK], kk_r[DK:P, :, 0:1, :])
            nc.vector.tensor_copy(kh2_r[0:DK, :, 1:2, DK:P], kk_r[0:DK, :, 1:2, :])

        def attention(b, q2):
            """Both heads at once via skew-packed block-diagonal K=128 matmuls.

            Denominators: even chunks ride the PE (ones2 stationary into a
            [2, qt] PSUM tile); odd chunks accumulate on the otherwise-idle
            DVE and join via one fp32 matmul at the end.  This drops the PE
            per-chunk cost from 6 to ~5 matmul streams, making the phase
            ScalarE(exp)-bound.
            """
            qs = qq_sb[:, b * L + q2 * 1024: b * L + (q2 + 1) * 1024]
            cp = ctxp.tile([P, 1024], F32, name="cp", tag="ctx")
            rs = rsp.tile([2, 1024], F32, name="rs", tag="rs")
            acc_o = smalls.tile([P, 1024], F32, name="acco", tag="acco", bufs=2)
            ex_prev = None
            for c in range(33):
                ex_cur = None
                if c < 32:
                    sc = mmp.tile([P, 1024], F32, name="sc", tag="mm")
                    for nb in range(2):
                        nc.tensor.matmul(
                            sc[:, nb * 512:(nb + 1) * 512],
                            lhsT=kh2[b][:, c * P:(c + 1) * P],
                            rhs=qs[:, nb * 512:(nb + 1) * 512],
                            start=True, stop=True, skip_group_check=True,
                        )
                    ex_cur = expp.tile([P, 1024], BF, name="ex", tag="exp")
                    nc.scalar.activation(ex_cur[:], sc[:], AF.Exp)
                if c >= 1:
                    cpv = c - 1
                    for nb in range(2):
                        sl = slice(nb * 512, (nb + 1) * 512)
                        nc.tensor.matmul(
                            cp[:, sl], lhsT=vh2[b][:, cpv * P:(cpv + 1) * P],
                            rhs=ex_prev[:, sl],
                            start=(cpv == 0), stop=(cpv == 31),
                            skip_group_check=True,
                        )
                    if cpv % 2 == 0:
                        for nb in range(2):
                            sl = slice(nb * 512, (nb + 1) * 512)
                            nc.tensor.matmul(
                                rs[:, sl], lhsT=ones2[:, 0:2],
                                rhs=ex_prev[:, sl],
                                start=(cpv == 0), stop=False,
                                skip_group_check=True,
                            )
                    elif cpv == 1:
                        nc.vector.tensor_copy(acc_o[:], ex_prev[:])
                    else:
                        nc.vector.tensor_add(acc_o[:], acc_o[:], ex_prev[:])
                ex_prev = ex_cur
            # fold the odd-chunk accumulator into rs (fp32 matmul, the
            # odd-parity ones selector swaps the head halves)
            for nb in range(2):
                sl = slice(nb * 512, (nb + 1) * 512)
                nc.tensor.matmul(
                    rs[:, sl], lhsT=ones2f[:], rhs=acc_o[:, sl],
                    start=False, stop=True, skip_group_check=True,
                )
            # normalize: reciprocal of the two denominator rows, broadcast
            # across partitions with a K=2 matmul, one DVE multiply.
            rsum = smalls.tile([2, 1024], F32, name="rsum", tag="rsum", bufs=2)
            nc.vector.tensor_copy(rsum[:], rs[:])
            rec = smalls.tile([2, 1024], F32, name="rec", tag="rec", bufs=2)
            nc.vector.reciprocal_approx_fast(rec[:], rsum[:])
            rec_bf = smalls.tile([2, 1024], BF, name="recbf", tag="recbf", bufs=2)
            nc.vector.tensor_copy(rec_bf[:], rec[:])
            bcs = rsp.tile([P, 1024], F32, name="bcs", tag="rs")
            for nb in range(2):
                sl = slice(nb * 512, (nb + 1) * 512)
                nc.tensor.matmul(
                    bcs[:, sl], lhsT=sel_sb[:], rhs=rec_bf[:, sl],
                    start=True, stop=True, skip_group_check=True,
                )
            craw = smalls.tile([P, 1024], BF, name="craw", tag="craw", bufs=2)
            nc.vector.tensor_copy(craw[:], cp[:])
            c0 = b * L + q2 * 1024
            nc.vector.tensor_mul(ctx_sb[:, c0:c0 + 1024], craw[:], bcs[:])

        def out_proj(b, q2, pool=None):
            # mid-section calls pass pool=ctxp: their PSUM tiles must not
            # rotate through mmp, whose slots are pinned by in-flight
            # projection tiles until the rope chain reads them
            pool = pool or mmp
            tag = "ctx" if pool is ctxp else "mm"
            ob = None
            for i, tb in enumerate(range(q2 * 8, q2 * 8 + 8)):
                t0 = b * L + tb * P
                if i % 4 == 0:
                    ob = outs.tile([P, 4 * D], BF, name="ob", tag="out")
                po = pool.tile([P, D], F32, name="po", tag=tag)
                for nb in range(2):
                    nc.tensor.matmul(
                        po[:, nb * 512:(nb + 1) * 512],
                        lhsT=ctx_sb[:, t0:t0 + P],
                        rhs=wo_sb[:, nb * 512:(nb + 1) * 512],
                        start=True, stop=True, skip_group_check=True,
                    )
                # evict on ScalarE: out_proj never overlaps attention exps
                # in this schedule, and DVE is the normalize/rope engine
                nc.scalar.activation(
                    ob[:, (i % 4) * D:(i % 4 + 1) * D], po[:], AF.Identity)
                if i % 4 == 3:
                    t00 = b * L + (tb - 3) * P
                    nc.sync.dma_start(
                        outp[t00:t00 + 4 * P, :].rearrange(
                            "(a p) d -> p a d", p=P),
                        ob.rearrange("p (a d) -> p a d", a=4))

        # ---------- program ----------
        # phase A order v -> k -> q: the LDW-bound v projection hides
        # under the k/q loads and the shortest post-load tail (q: rope
        # only) lands right before attention
        wv_sb = load_w("wv_sb", wvT)
        xv0 = load_x(vT, 0)
        proj_v(xv0, 0)
        dummies(10)
        wk_sb = load_w("wk_sb", wkT)
        bk_sb = load_b("bk_sb", bk_d)
        bkr_sb = load_b("bkr_sb", bkr_d)
        ck_sb = load_c("ck_sb", cos_k)
        sk_sb = load_c("sk_sb", sin_k)
        xk0 = load_x(kT, 0)
        proj_compute(xk0, wk_sb, bk_sb, bkr_sb, 0, kk_sb, ck_sb, sk_sb)
        build_kh2(0)
        dummies(16)
        bqr_sb = load_b("bqr_sb", bqr_d)
        cq_sb = load_c("cq_sb", cos_q)
        sq_sb = load_c("sq_sb", sin_q)
        xq0 = load_x(qT, 0)
        proj_compute(xq0, wq_sb, bq_sb, bqr_sb, 0, qq_sb, cq_sb, sq_sb)
        # keep the PE busy (and the clock gate open) across the rope-q
        # chain so attention(0,0) starts warm
        dummies(24)
        wo_sb = const.tile([P, D], BF)
        nc.sync.dma_start(wo_sb[:], woT[:])
        # queue batch-1 input DMA now: it streams on the sync queue while
        # the PE runs batch-0 attention; batch-1 projections are slotted
        # between attention phases so stage buffers keep recycling
        xv1 = load_x(vT, 1)
        xk1 = load_x(kT, 1)
        xq1 = load_x(qT, 1)
        attention(0, 0)
        proj_v(xv1, 1)
        attention(0, 1)
        proj_compute(xk1, wk_sb, bk_sb, bkr_sb, 1, kk_sb, ck_sb, sk_sb)
        build_kh2(1)
        out_proj(0, 0, pool=ctxp)
        proj_compute(xq1, wq_sb, bq_sb, bqr_sb, 1, qq_sb, cq_sb, sq_sb)
        out_proj(0, 1, pool=ctxp)
        attention(1, 0)
        attention(1, 1)
        out_proj(1, 0)
        out_proj(1, 1)

    return nc


# Per-head dim permutation making rotate-half a within-32-quadrant
# 16-swap (the device's stream_shuffle): [0:16, 32:48, 16:32, 48:64].
# q and k are permuted identically so attention scores are unchanged.
_PERM64 = np.concatenate([np.arange(0, 16), np.arange(32, 48),
                          np.arange(16, 32), np.arange(48, 64)])
_PERM128 = np.concatenate([_PERM64, _PERM64 + DK])
# the shuffle the device applies: out[j] = in[(j//32)*32 + (j%32+16)%32]
_SHUF128 = np.array([(j // 32) * 32 + (j % 32 + 16) % 32 for j in range(P)])


def _rope_tables():
    """Host-built RoPE tables, transposed to [d, t] (unique 64 rows).

    sin is sign-folded for the rotate-half convention; q tables carry the
    1/sqrt(dk) attention scale.  Rows are dim-permuted to match the
    on-device shuffle layout; the kernel duplicates rows onto partitions
    64:128 on-chip.
    """
    inv_freq = 1.0 / (ROPE_BASE ** (np.arange(0, DK, 2, dtype=np.float64) / DK))
    t = np.arange(L, dtype=np.float64)
    ang = np.outer(t, inv_freq)               # [L, 32]
    emb = np.concatenate([ang, ang], axis=1)  # [L, 64]
    cos = np.cos(emb).T.astype(np.float32)    # [64, L]
    sin = np.sin(emb).T.astype(np.float32)
    sin_folded = sin.copy()
    sin_folded[:32] *= -1.0
    cos = cos[_PERM64]
    sin_folded = sin_folded[_PERM64]
    scale = 1.0 / np.sqrt(DK)
    bf = ml_dtypes.bfloat16
    return (
        (cos * scale).astype(bf), (sin_folded * scale).astype(bf),
        cos.astype(bf), sin_folded.astype(bf),
    )


_NC_CACHE = {}


def _get_nc():
    if "nc" not in _NC_CACHE:
        nc = build_nc()
        nc.finalize()
        _NC_CACHE["nc"] = nc
    return _NC_CACHE["nc"]


def _in_maps(q, k, v, Wq, bq, Wk, bk, Wv, Wo):
    bf = ml_dtypes.bfloat16
    qT = np.ascontiguousarray(q.reshape(TOK, D).T).astype(bf)
    kT = np.ascontiguousarray(k.reshape(TOK, D).T).astype(bf)
    vT = np.ascontiguousarray(v.reshape(TOK, D).T).astype(bf)
    cos_q, sin_q, cos_k, sin_k = _rope_tables()
    # even-chunk selectors: rows 0:64 = h0, rows 64:128 = h1
    # odd-chunk selectors (skewed): rows 0:64 = h1, rows 64:128 = h0
    ones2 = np.zeros((P, 4), bf)
    ones2[0:DK, 0] = 1
    ones2[DK:P, 1] = 1
    ones2[DK:P, 2] = 1
    ones2[0:DK, 3] = 1
    ones2f = np.ascontiguousarray(ones2[:, 2:4]).astype(np.float32)
    sel = np.zeros((2, P), bf)
    sel[0, 0:DK] = 1
    sel[1, DK:P] = 1
    in_maps = []
    for c in range(NCORES):
        hs = slice(c * PD, (c + 1) * PD)
        # q/k weight rows and biases carry the rope-shuffle dim permutation
        wq_c = np.asarray(Wq[hs, :])[_PERM128]
        wk_c = np.asarray(Wk[hs, :])[_PERM128]
        bq_c = np.asarray(bq[hs], np.float32)[_PERM128]
        bk_c = np.asarray(bk[hs], np.float32)[_PERM128]
        in_maps.append({
            "qT": qT, "kT": kT, "vT": vT,
            "wqT": np.ascontiguousarray(wq_c.T).astype(bf),
            "wkT": np.ascontiguousarray(wk_c.T).astype(bf),
            "wvT": np.ascontiguousarray(Wv[hs, :].T).astype(bf),
            "woT": np.ascontiguousarray(Wo[:, hs].T).astype(bf),
            "bq": bq_c.reshape(PD, 1),
            "bk": bk_c.reshape(PD, 1),
            "bqr": bq_c[_SHUF128].reshape(PD, 1).copy(),
            "bkr": bk_c[_SHUF128].reshape(PD, 1).copy(),
            "cos_q": cos_q, "sin_q": sin_q, "cos_k": cos_k, "sin_k": sin_k,
            "ones2": ones2, "ones2f": ones2f, "sel": sel,
        })
    return in_maps


def kernel(q, k, v, Wq, bq, Wk, bk, Wv, bv, Wo, bo):
    assert q.shape == (B, L, D) and k.shape == (B, L, D) and v.shape == (B, L, D)
    in_maps = _in_maps(q, k, v, Wq, bq, Wk, bk, Wv, Wo)
    nc = _get_nc()
    res = run_bass_kernel_spmd(nc, in_maps, list(range(NCORES)))
    out = np.zeros((TOK, D), np.float64)
    for r in res.results:
        out += r["outp"].astype(np.float64)
    # bv never touches the device: since the attention weights sum to 1,
    # its contribution is the constant vector Wo @ bv
    out += np.asarray(bo, np.float64)[None, :]
    out += (np.asarray(Wo, np.float64) @ np.asarray(bv, np.float64))[None, :]
    return out.astype(np.float32).reshape(B, L, D)
